# revision 8
# baseline (speedup 1.0000x reference)
"""Trainium2 Bass kernel for nn_EnhancedFractionalPINO.

Pipeline (per core, batch-parallel over 8 NeuronCores, 32 batches/core):
  1. f = Re(fft2(x)) per 64x64 image via cosine/sine DFT matmuls
  2. GL fractional derivative = truncated causal conv (KTAPS taps) over the
     globally-flattened signal, as Toeplitz-block matmuls (halo image passed
     from the previous core's batch range; zeros for core 0)
  3. spectral_operator + neural_operator MLPs as PE matmuls (fp32r)
  4. out = Re(ifft2(proc)) via the same DFT-matmul machinery

Weights are replicated across cores; activations stay SBUF-resident.
"""

import numpy as np

import concourse.bass as bass
import concourse.mybir as mybir
import concourse.tile as tile
from concourse import bacc
from concourse.bass_utils import run_bass_kernel_spmd

F32 = mybir.dt.float32
F32R = mybir.dt.float32r
AF = mybir.ActivationFunctionType
ALU = mybir.AluOpType

B, C, H, W = 256, 3, 64, 64
MODES = C * H * W              # 12288
ALPHA = 0.5
NTOT = B * MODES               # 3145728 flattened samples
NCORE = 8
BS = B // NCORE                # 32 batches per core
NIMG = BS * C                  # 96 images per core
KTAPS = 512                    # truncated GL taps (4 chunks of 128)
LCH = 128                      # conv chunk length
NCH = BS * MODES // LCH        # 3072 output chunks per core
NBLK = NCH // 512              # 6 conv blocks of 512 chunks


# ---------------------------------------------------------------- host consts
def _host_constants():
    jk = np.outer(np.arange(64), np.arange(64)).astype(np.float64)
    Cm = np.cos(2 * np.pi * jk / 64).astype(np.float32)
    Sm = np.sin(2 * np.pi * jk / 64).astype(np.float32)

    j = np.arange(1, KTAPS, dtype=np.float64)
    w = np.concatenate([[1.0], np.cumprod((j - 1.0 - ALPHA) / j)])
    w = (w * np.float64(1.0 / (NTOT - 1)) ** (-ALPHA)).astype(np.float32)

    # Tst[d][t, tau] = w[128*d + tau - t]  (lhsT layout of the Toeplitz blocks)
    idx = 128 * np.arange(4)[:, None, None] \
        + np.arange(128)[None, None, :] - np.arange(128)[None, :, None]
    Tst = np.where((idx >= 0) & (idx < KTAPS), w[np.clip(idx, 0, KTAPS - 1)], 0.0)
    Tst = np.ascontiguousarray(Tst.astype(np.float32))

    CS1 = np.ascontiguousarray(np.concatenate([Cm, Sm], axis=0))          # [128,64]
    CmS2 = np.ascontiguousarray(np.concatenate([Cm, -Sm], axis=0))        # [128,64]
    CmS2i = np.ascontiguousarray(CmS2 / 4096.0)
    IDN32 = np.eye(32, dtype=np.float32)
    return CS1, CmS2, CmS2i, Tst, IDN32


def _prep_weights(Ws1, bs1, Ws2, bs2, Wn1, bn1, Wn2, bn2, Wn3, bn3):
    W1t = np.ascontiguousarray(Ws1.reshape(96, 128, 512))
    W2t = np.ascontiguousarray(Ws2.reshape(4, 128, 96, 128).transpose(2, 1, 0, 3))
    W3t = np.ascontiguousarray(Wn1.reshape(96, 128, 512))
    W4t = np.ascontiguousarray(Wn2.reshape(4, 128, 4, 128).transpose(2, 1, 0, 3))
    W5t = np.ascontiguousarray(Wn3.reshape(4, 128, 192, 64).transpose(2, 1, 0, 3))
    b1r = np.ascontiguousarray(bs1.reshape(1, 512))
    b2T = np.ascontiguousarray(bs2.reshape(96, 128).T)    # [128, 96]
    b3r = np.ascontiguousarray(bn1.reshape(1, 512))
    b4T = np.ascontiguousarray(bn2.reshape(4, 128).T)     # [128, 4]
    b5T = np.ascontiguousarray(bn3.reshape(192, 64).T)    # [64, 192]
    return W1t, b1r, W2t, b2T, W3t, b3r, W4t, b4T, W5t, b5T


# ---------------------------------------------------------------- bass module
_NC_CACHE = None





def _build_nc():
    nc = bacc.Bacc("TRN2", target_bir_lowering=False, debug=False,
                   num_devices=NCORE)

    d_ximgs = nc.dram_tensor("ximgs", (NIMG + 1, 64, 64), F32R, kind="ExternalInput")
    d_cs1 = nc.dram_tensor("cs1", (128, 64), F32R, kind="ExternalInput")
    d_cms2 = nc.dram_tensor("cms2", (128, 64), F32R, kind="ExternalInput")
    d_cms2i = nc.dram_tensor("cms2i", (128, 64), F32R, kind="ExternalInput")
    d_tst = nc.dram_tensor("tst", (4, 128, 128), F32R, kind="ExternalInput")
    d_idn = nc.dram_tensor("idn32", (32, 32), F32R, kind="ExternalInput")
    d_ones = nc.dram_tensor("ones1", (1, 32), F32R, kind="ExternalInput")
    d_zb = nc.dram_tensor("zb128", (128, 128), F32R, kind="ExternalInput")
    d_w1 = nc.dram_tensor("w1t", (96, 128, 512), F32R, kind="ExternalInput")
    d_b1 = nc.dram_tensor("b1r", (1, 512), F32R, kind="ExternalInput")
    d_w2 = nc.dram_tensor("w2t", (96, 128, 4, 128), F32R, kind="ExternalInput")
    d_b2 = nc.dram_tensor("b2t", (128, 96), F32, kind="ExternalInput")
    d_w3 = nc.dram_tensor("w3t", (96, 128, 512), F32R, kind="ExternalInput")
    d_b3 = nc.dram_tensor("b3r", (1, 512), F32R, kind="ExternalInput")
    d_w4 = nc.dram_tensor("w4t", (4, 128, 4, 128), F32R, kind="ExternalInput")
    d_b4 = nc.dram_tensor("b4t", (128, 4), F32, kind="ExternalInput")
    d_w5 = nc.dram_tensor("w5t", (192, 128, 4, 64), F32R, kind="ExternalInput")
    d_b5 = nc.dram_tensor("b5t", (64, 192), F32, kind="ExternalInput")
    d_out = nc.dram_tensor("out", (BS, C, 64, 64), F32, kind="ExternalOutput")

    with tile.TileContext(nc) as tc:
        with tc.tile_pool(name="cpool", bufs=1) as cpool, \
             tc.tile_pool(name="bigpool", bufs=1) as bigpool:
            # ---- constants into SBUF
            cs1 = cpool.tile([128, 64], F32R, tag="cs1")
            cms2 = cpool.tile([128, 64], F32R, tag="cms2")
            cms2i = cpool.tile([128, 64], F32R, tag="cms2i")
            tsb = cpool.tile([128, 4, 128], F32R, tag="tsb")
            idn = cpool.tile([32, 32], F32R, tag="idn")
            ones1 = cpool.tile([1, 32], F32R, tag="ones1")
            b1s = cpool.tile([1, 512], F32R, tag="b1s")
            b2s = cpool.tile([128, 96], F32, tag="b2s")
            b3s = cpool.tile([1, 512], F32R, tag="b3s")
            b4s = cpool.tile([128, 4], F32, tag="b4s")
            b5s = cpool.tile([64, 192], F32, tag="b5s")
            nc.sync.dma_start(cs1[:], d_cs1[:])
            nc.sync.dma_start(cms2[:], d_cms2[:])
            nc.sync.dma_start(cms2i[:], d_cms2i[:])
            for d in range(4):
                nc.sync.dma_start(tsb[:, d, :], d_tst[d])
            nc.sync.dma_start(idn[:], d_idn[:])
            nc.sync.dma_start(ones1[:], d_ones[:])
            nc.sync.dma_start(b1s[:], d_b1[:])
            nc.sync.dma_start(b2s[:], d_b2[:])
            nc.sync.dma_start(b3s[:], d_b3[:])
            nc.sync.dma_start(b4s[:], d_b4[:])
            nc.sync.dma_start(b5s[:], d_b5[:])

            # ---- persistent activation tiles
            fbuf = bigpool.tile([128, 4 + NCH], F32R, tag="fbuf")
            frlin = bigpool.tile([128, NCH], F32R, tag="frlin")
            specT = bigpool.tile([128, 96, BS], F32R, tag="specT")
            procT = bigpool.tile([128, 192, BS], F32R, tag="procT")
            hT = bigpool.tile([128, 4, BS], F32R, tag="hT")
            h1T = bigpool.tile([128, 4, BS], F32R, tag="h1T")
            h2T = bigpool.tile([128, 4, BS], F32R, tag="h2T")
            h_sb = bigpool.tile([32, 512], F32R, tag="h_sb")
            h1_sb = bigpool.tile([32, 512], F32R, tag="h1_sb")
            bds = [bigpool.tile([128, 128], F32R, tag=f"bd{i}", name=f"bd{i}")
                   for i in range(2)]
            bdps = [bigpool.tile([128, 128], F32R, tag=f"bdp{i}", name=f"bdp{i}")
                    for i in range(2)]
            for t in bds + bdps:
                nc.sync.dma_start(t[:], d_zb[:])

            # ================= phase 1: fft2 (97 images incl. halo) =========
            with tc.tile_pool(name="gpool", bufs=3) as gpool, \
                 tc.tile_pool(name="ps1p", bufs=3, space="PSUM") as ps1p, \
                 tc.tile_pool(name="ps2p", bufs=3, space="PSUM") as ps2p:
                for i in range(NIMG + 1):
                    bd = bds[i % 2]
                    nc.sync.dma_start(bd[0:64, 0:64], d_ximgs[i])
                    nc.sync.dma_start(bd[64:128, 64:128], d_ximgs[i])
                    ps1 = ps1p.tile([128, 64], F32, tag="ps1")
                    nc.tensor.matmul(ps1[:], (bd[:]), (cs1[:]),
                                     start=True, stop=True)
                    g1 = gpool.tile([128, 64], F32R, tag="g1")
                    nc.scalar.copy(g1[:], ps1[:])
                    ps2 = ps2p.tile([64, 64], F32, tag="ps2")
                    nc.tensor.matmul(ps2[:], (cms2[:]), (g1[:]),
                                     start=True, stop=True)
                    p2v = ps2.rearrange("p (k two) -> p k two", two=2)
                    if i == 0:
                        nc.vector.tensor_copy(fbuf[0:64, 0:4], p2v[:, 28:32, 0])
                        nc.vector.tensor_copy(fbuf[64:128, 0:4], p2v[:, 28:32, 1])
                    else:
                        base = 4 + (i - 1) * 32
                        nc.vector.tensor_copy(fbuf[0:64, base:base + 32],
                                              p2v[:, :, 0])
                        nc.vector.tensor_copy(fbuf[64:128, base:base + 32],
                                              p2v[:, :, 1])

            # ================= phase 2: conv ================================
            with tc.tile_pool(name="pscv", bufs=1, space="PSUM") as pscv:
                psc = [pscv.tile([128, 512], F32, tag=f"psc{i}", name=f"psc{i}")
                       for i in range(NBLK)]
                for d in range(4):
                    for blk in range(NBLK):
                        o = 4 + blk * 512 - d
                        nc.tensor.matmul(psc[blk][:], (tsb[:, d, :]),
                                         (fbuf[:, o:o + 512]),
                                         start=(d == 0), stop=(d == 3))
                for blk in range(NBLK):
                    nc.vector.tensor_copy(frlin[:, blk * 512:(blk + 1) * 512],
                                          psc[blk][:])

            frl3 = frlin.rearrange("p (b k) -> p b k", b=BS)

            # ================= phase 3/5 helper: acts-stationary layer ======
            def big_layer(src_blk, d_w, bias_row, out_sb, outT):
                """out_sb[32,512] = relu(sum_K src_blk(K).T @ W[K] + bias);
                then transpose into outT [128, 4, 32]."""
                with tc.tile_pool(name="wp", bufs=3) as wp, \
                     tc.tile_pool(name="psm", bufs=1, space="PSUM") as psm, \
                     tc.tile_pool(name="pst", bufs=2, space="PSUM") as pst:
                    acc = psm.tile([32, 512], F32, tag="acc")
                    for K in range(96):
                        wt = wp.tile([128, 512], F32R, tag="wt")
                        nc.sync.dma_start(wt[:], d_w[K])
                        nc.tensor.matmul(acc[:], (src_blk(K)), (wt[:]),
                                         start=(K == 0), stop=False)
                    nc.tensor.matmul(acc[:], (ones1[:]), (bias_row[:]),
                                     start=False, stop=True)
                    nc.scalar.activation(out_sb[:], acc[:], AF.Relu)
                    for fb in range(4):
                        pt = pst.tile([128, 32], F32R, tag="pt")
                        nc.tensor.transpose(pt[:], out_sb[:, fb * 128:(fb + 1) * 128],
                                            idn[:])
                        nc.vector.tensor_copy(outT[:, fb, :], pt[:])

            # ================= phase 3: L1 ==================================
            big_layer(lambda K: frl3[:, :, K], d_w1, b1s, h_sb, hT)

            # ================= phase 4: L2 (weights-stationary) =============
            with tc.tile_pool(name="wp2", bufs=3) as wp2, \
                 tc.tile_pool(name="ps2m", bufs=4, space="PSUM") as ps2m:
                for mb in range(96):
                    wt = wp2.tile([128, 4, 128], F32R, tag="w2")
                    nc.sync.dma_start(wt[:], d_w2[mb])
                    acc = ps2m.tile([128, 32], F32, tag="acc2")
                    for fb in range(4):
                        nc.tensor.matmul(acc[:], (wt[:, fb, :]), (hT[:, fb, :]),
                                         start=(fb == 0), stop=(fb == 3))
                    nc.vector.tensor_scalar_add(specT[:, mb, :], acc[:],
                                                b2s[:, mb:mb + 1])

            # ================= phase 5: L3 ==================================
            big_layer(lambda K: specT[:, K, :], d_w3, b3s, h1_sb, h1T)

            # ================= phase 6: L4 ==================================
            with tc.tile_pool(name="wp4", bufs=2) as wp4, \
                 tc.tile_pool(name="ps4m", bufs=2, space="PSUM") as ps4m:
                for f2b in range(4):
                    wt = wp4.tile([128, 4, 128], F32R, tag="w4")
                    nc.sync.dma_start(wt[:], d_w4[f2b])
                    acc = ps4m.tile([128, 32], F32, tag="acc4")
                    for fb in range(4):
                        nc.tensor.matmul(acc[:], (wt[:, fb, :]), (h1T[:, fb, :]),
                                         start=(fb == 0), stop=(fb == 3))
                    nc.vector.tensor_scalar(h2T[:, f2b, :], acc[:],
                                            b4s[:, f2b:f2b + 1], 0.0,
                                            ALU.add, ALU.max)

            # ================= phase 7: L5 ==================================
            with tc.tile_pool(name="wp5", bufs=3) as wp5, \
                 tc.tile_pool(name="ps5m", bufs=4, space="PSUM") as ps5m:
                for mb in range(192):
                    wt = wp5.tile([128, 4, 64], F32R, tag="w5")
                    nc.sync.dma_start(wt[:], d_w5[mb])
                    acc = ps5m.tile([64, 32], F32, tag="acc5")
                    for fb in range(4):
                        nc.tensor.matmul(acc[:], (wt[:, fb, :]), (h2T[:, fb, :]),
                                         start=(fb == 0), stop=(fb == 3))
                    nc.vector.tensor_scalar_add(procT[0:64, mb, :], acc[:],
                                                b5s[:, mb:mb + 1])
            # duplicate to lower partitions for blockdiag builds
            nc.sync.dma_start(procT[64:128, :, :], procT[0:64, :, :])

            # ================= phase 8: ifft2 ===============================
            with tc.tile_pool(name="gpi", bufs=3) as gpi, \
                 tc.tile_pool(name="owp", bufs=2) as owp, \
                 tc.tile_pool(name="ps1i", bufs=3, space="PSUM") as ps1i, \
                 tc.tile_pool(name="ps2i", bufs=3, space="PSUM") as ps2i:
                for b in range(BS):
                    owide = owp.tile([64, 192], F32, tag="owide")
                    for c in range(C):
                        bdp = bdps[(b * C + c) % 2]
                        nc.vector.tensor_copy(bdp[0:64, 0:64],
                                              procT[0:64, c * 64:(c + 1) * 64, b])
                        nc.vector.tensor_copy(bdp[64:128, 64:128],
                                              procT[64:128, c * 64:(c + 1) * 64, b])
                        ps1 = ps1i.tile([128, 64], F32, tag="p1i")
                        nc.tensor.matmul(ps1[:], (bdp[:]), (cs1[:]),
                                         start=True, stop=True)
                        g1 = gpi.tile([128, 64], F32R, tag="g1i")
                        nc.scalar.copy(g1[:], ps1[:])
                        ps2 = ps2i.tile([64, 64], F32, tag="p2i")
                        nc.tensor.matmul(ps2[:], (cms2i[:]), (g1[:]),
                                         start=True, stop=True)
                        nc.scalar.copy(owide[:, c * 64:(c + 1) * 64], ps2[:])
                    for c in range(C):
                        nc.sync.dma_start(d_out[b, c], owide[:, c * 64:(c + 1) * 64])

    nc.compile()
    return nc


def _get_nc():
    global _NC_CACHE
    if _NC_CACHE is None:
        _NC_CACHE = _build_nc()
    return _NC_CACHE


def _make_in_maps(x, Ws1, bs1, Ws2, bs2, Wn1, bn1, Wn2, bn2, Wn3, bn3):
    CS1, CmS2, CmS2i, Tst, IDN32 = _host_constants()
    W1t, b1r, W2t, b2T, W3t, b3r, W4t, b4T, W5t, b5T = _prep_weights(
        Ws1, bs1, Ws2, bs2, Wn1, bn1, Wn2, bn2, Wn3, bn3)
    shared = {
        "cs1": CS1, "cms2": CmS2, "cms2i": CmS2i, "tst": Tst, "idn32": IDN32,
        "ones1": np.ones((1, 32), np.float32),
        "zb128": np.zeros((128, 128), np.float32),
        "w1t": W1t, "b1r": b1r, "w2t": W2t, "b2t": b2T,
        "w3t": W3t, "b3r": b3r, "w4t": W4t, "b4t": b4T,
        "w5t": W5t, "b5t": b5T,
    }
    in_maps = []
    for g in range(NCORE):
        if g == 0:
            halo = np.zeros((1, 64, 64), np.float32)
        else:
            halo = x[g * BS - 1, 2][None]
        ximgs = np.ascontiguousarray(
            np.concatenate([halo, x[g * BS:(g + 1) * BS].reshape(NIMG, 64, 64)]))
        in_maps.append({"ximgs": ximgs, **shared})
    return in_maps


def kernel(**inputs):
    x = np.ascontiguousarray(inputs["x"], dtype=np.float32)
    nc = _get_nc()
    in_maps = _make_in_maps(
        x, inputs["Ws1"], inputs["bs1"], inputs["Ws2"], inputs["bs2"],
        inputs["Wn1"], inputs["bn1"], inputs["Wn2"], inputs["bn2"],
        inputs["Wn3"], inputs["bn3"])
    res = run_bass_kernel_spmd(nc, in_maps, list(range(NCORE)))
    out = np.empty((B, C, H, W), np.float32)
    for g in range(NCORE):
        out[g * BS:(g + 1) * BS] = res.results[g]["out"]
    return out


# revision 25
# speedup vs baseline: 520.1417x; 520.1417x over previous
"""Trainium2 Bass kernel for nn_EnhancedFractionalPINO.

Pipeline (per core, batch-parallel over 8 NeuronCores, 32 batches/core):
  1. f = Re(fft2(x)) per 64x64 image via cosine/sine DFT matmuls:
     m1: per image, lhsT = image, rhs = [C | S] -> [x^T C | x^T S];
     m2: per 8-image group, two const-stationary matmuls with strided rhs
     -> A^T = C x^T C - S x^T S for all 8 images in one psum tile.
  2. GL fractional derivative = truncated causal conv (KTAPS taps) over the
     globally-flattened signal, as Toeplitz-block matmuls (halo image passed
     from the previous core's batch range; zeros for core 0). The h^-alpha
     scale is folded into Ws1 so everything stays in fp16 range.
  3. spectral_operator + neural_operator MLPs as fp16 PE matmuls with a
     positive rescaling chain (LAM_*) keeping activations in fp16 range;
     activations-stationary, PE transposes between layers.
  4. out = Re(ifft2(proc)) via the same DFT-matmul machinery (scales folded
     into the second-stage constants).

Weights are replicated across cores; activations stay SBUF-resident.
"""

import numpy as np

import concourse.bass as bass
import concourse.mybir as mybir
import concourse.tile as tile
from concourse import bacc
from concourse.bass_utils import run_bass_kernel_spmd

F32 = mybir.dt.float32
F16 = mybir.dt.float16
AF = mybir.ActivationFunctionType

B, C, H, W = 256, 3, 64, 64
MODES = C * H * W              # 12288
ALPHA = 0.5
NTOT = B * MODES               # 3145728 flattened samples
NCORE = 8
BS = B // NCORE                # 32 batches per core
NIMG = BS * C                  # 96 images per core
NSLOT = NIMG + 2               # halo + 96 images + zero pad
KTAPS = 512                    # truncated GL taps (4 chunks of 128)
NCH = BS * MODES // 128        # 3072 output chunks per core
NBLK = NCH // 512              # 6 conv blocks of 512 chunks

# fp16 activation rescaling chain (see mirror3 validation)
LAM_H, LAM_S, LAM_1, LAM_2, LAM_P = 16.0, 8.0, 4.0, 4.0, 4.0


# ---------------------------------------------------------------- host consts
def _host_constants():
    jk = np.outer(np.arange(64), np.arange(64)).astype(np.float64)
    Cm = np.cos(2 * np.pi * jk / 64)
    Sm = np.sin(2 * np.pi * jk / 64)

    j = np.arange(1, KTAPS, dtype=np.float64)
    w = np.concatenate([[1.0], np.cumprod((j - 1.0 - ALPHA) / j)])

    # Tst[d][t, tau] = w[128*d + tau - t]  (lhsT layout of the Toeplitz blocks)
    idx = 128 * np.arange(4)[:, None, None] \
        + np.arange(128)[None, None, :] - np.arange(128)[None, :, None]
    Tst = np.where((idx >= 0) & (idx < KTAPS), w[np.clip(idx, 0, KTAPS - 1)], 0.0)

    f16 = lambda a: np.ascontiguousarray(a, dtype=np.float16)
    return {
        "cswi": f16(np.concatenate([Cm, Sm], axis=1)),     # [64, 128]
        "cmf": f16(Cm),                                    # [64, 64]
        "msf": f16(-Sm),
        "cmi": f16(Cm * (LAM_P / 4096.0)),
        "smi": f16(-Sm * (LAM_P / 4096.0)),
        "tst": f16(Tst),
        "idn32": f16(np.eye(32)),
        "ones1": f16(np.ones((1, 32))),
    }


def _prep_weights(Ws1, bs1, Ws2, bs2, Wn1, bn1, Wn2, bn2, Wn3, bn3):
    s = float(np.float64(1.0 / (NTOT - 1)) ** (-ALPHA))
    f16 = lambda a: np.ascontiguousarray(a, dtype=np.float16)
    W1 = (Ws1.astype(np.float64) * (s / LAM_H)).astype(np.float32)
    W2 = Ws2 * np.float32(LAM_H / LAM_S)
    W3 = Wn1 * np.float32(LAM_S / LAM_1)
    W4 = Wn2 * np.float32(LAM_1 / LAM_2)
    W5 = Wn3 * np.float32(LAM_2 / LAM_P)
    return {
        "w1t": f16(W1.reshape(24, 4, 128, 512).transpose(0, 2, 1, 3)),
        "w2r": f16(W2.reshape(4, 128, 12, 1024).transpose(2, 1, 0, 3)),
        "w3t": f16(W3.reshape(24, 4, 128, 512).transpose(0, 2, 1, 3)),
        "w4t": f16(W4.reshape(4, 128, 4, 128).transpose(2, 1, 0, 3)
                   .reshape(4, 128, 512)),
        "w5r": f16(W5.reshape(4, 128, 12, 1024).transpose(2, 1, 0, 3)),
        "b1r": f16((bs1 / LAM_H).reshape(1, 512)),
        "b2r": f16((bs2 / LAM_S).reshape(1, MODES)),
        "b3r": f16((bn1 / LAM_1).reshape(1, 512)),
        "b4t": np.ascontiguousarray((bn2 / LAM_2).reshape(4, 128).T,
                                    dtype=np.float32),     # [128, 4]
        "b5r": f16((bn3 / LAM_P).reshape(1, MODES)),
    }


# ---------------------------------------------------------------- bass module
_NC_CACHE = None


def _build_nc():
    nc = bacc.Bacc("TRN2", target_bir_lowering=False, debug=False,
                   num_devices=NCORE)

    def din(name, shape, dt=F16):
        return nc.dram_tensor(name, shape, dt, kind="ExternalInput")

    d_x = din("ximgs", (NSLOT, 64, 64))
    d_cswi = din("cswi", (64, 128))
    d_cmf = din("cmf", (64, 64))
    d_msf = din("msf", (64, 64))
    d_cmi = din("cmi", (64, 64))
    d_smi = din("smi", (64, 64))
    d_tst = din("tst", (4, 128, 128))
    d_idn = din("idn32", (32, 32))
    d_ones = din("ones1", (1, 32))
    d_w1 = din("w1t", (24, 128, 4, 512))
    d_w2 = din("w2r", (12, 128, 4, 1024))
    d_w3 = din("w3t", (24, 128, 4, 512))
    d_w4 = din("w4t", (4, 128, 512))
    d_w5 = din("w5r", (12, 128, 4, 1024))
    d_b1 = din("b1r", (1, 512))
    d_b2 = din("b2r", (1, MODES))
    d_b3 = din("b3r", (1, 512))
    d_b4 = nc.dram_tensor("b4t", (128, 4), F32, kind="ExternalInput")
    d_b5 = din("b5r", (1, MODES))
    d_out = nc.dram_tensor("out", (BS, C, 64, 64), F32, kind="ExternalOutput")

    with tile.TileContext(nc) as tc:
        with tc.tile_pool(name="cpool", bufs=1) as cpool, \
             tc.tile_pool(name="bigpool", bufs=1) as bigpool:
            # ---- constants into SBUF
            cswi = cpool.tile([64, 128], F16, tag="cswi")
            cmf = cpool.tile([64, 64], F16, tag="cmf")
            msf = cpool.tile([64, 64], F16, tag="msf")
            cmi = cpool.tile([64, 64], F16, tag="cmi")
            smi = cpool.tile([64, 64], F16, tag="smi")
            tsb = cpool.tile([128, 4, 128], F16, tag="tsb")
            idn = cpool.tile([32, 32], F16, tag="idn")
            ones1 = cpool.tile([1, 32], F16, tag="ones1")
            b1s = cpool.tile([1, 512], F16, tag="b1s")
            b3s = cpool.tile([1, 512], F16, tag="b3s")
            b4s = cpool.tile([128, 4], F32, tag="b4s")
            bbig = cpool.tile([1, MODES], F16, tag="bbig")  # b2 then b5
            for t, d in ((cswi, d_cswi), (cmf, d_cmf), (msf, d_msf),
                         (cmi, d_cmi), (smi, d_smi), (idn, d_idn),
                         (ones1, d_ones), (b1s, d_b1), (b3s, d_b3),
                         (b4s, d_b4)):
                nc.sync.dma_start(t[:], d[:])
            nc.sync.dma_start(tsb[:], d_tst.rearrange("d p k -> p d k"))

            # ---- persistent activation tiles
            fbuf = bigpool.tile([128, 4 + NCH + 64], F16, tag="fbuf")
            frlin = bigpool.tile([128, NCH], F16, tag="frlin")
            specT = bigpool.tile([128, 96, BS], F16, tag="specT")
            procT = bigpool.tile([64, 192, BS], F16, tag="procT")
            hT = bigpool.tile([128, 4, BS], F16, tag="hT")
            h1T = bigpool.tile([128, 4, BS], F16, tag="h1T")
            h2T = bigpool.tile([128, 4, BS], F16, tag="h2T")
            h_sb = bigpool.tile([32, 512], F16, tag="h_sb")
            h1_sb = bigpool.tile([32, 512], F16, tag="h1_sb")

            # ========== phase 1: fft2 (per-image m1, 8-wide m2) =============
            with tc.tile_pool(name="xpool", bufs=1) as xpool, \
                 tc.tile_pool(name="gpool", bufs=3) as gpool, \
                 tc.tile_pool(name="ps1p", bufs=2, space="PSUM") as ps1p, \
                 tc.tile_pool(name="ps2p", bufs=2, space="PSUM") as ps2p:
                xall = xpool.tile([64, NSLOT, 64], F16, tag="xall")
                for ch in range(4):
                    q0 = (NSLOT * ch) // 4
                    q1 = (NSLOT * (ch + 1)) // 4
                    nc.sync.dma_start(
                        xall[:, q0:q1, :],
                        d_x[q0:q1].rearrange("q p k -> p q k"))
                for grp in range(25):
                    n = 4 if grp < 24 else 2
                    psA = ps1p.tile([64, 512], F32, tag="psA")
                    for t in range(n):
                        i = grp * 4 + t
                        nc.tensor.matmul(psA[:, t * 128:(t + 1) * 128],
                                         xall[:, i, :], cswi[:],
                                         start=True, stop=True)
                    g1w = gpool.tile([64, 4, 128], F16, tag="g1w")
                    nc.scalar.copy(g1w[:, 0:n, :].rearrange("p a k -> p (a k)"),
                                   psA[:, 0:n * 128])
                    ps2 = ps2p.tile([64, 256], F32, tag="ps2")
                    nc.tensor.matmul(ps2[:, 0:n * 64], cmf[:],
                                     g1w[:, 0:n, 0:64], start=True, stop=False)
                    nc.tensor.matmul(ps2[:, 0:n * 64], msf[:],
                                     g1w[:, 0:n, 64:128], start=False, stop=True)
                    p2v = ps2.rearrange("p (k two) -> p k two", two=2)
                    if grp == 0:
                        # halo image: last 4 chunk-cols; imgs 1..3 -> cols 4:100
                        nc.vector.tensor_copy(fbuf[0:64, 0:4], p2v[:, 28:32, 0])
                        nc.vector.tensor_copy(fbuf[64:128, 0:4], p2v[:, 28:32, 1])
                        nc.vector.tensor_copy(fbuf[0:64, 4:100], p2v[:, 32:128, 0])
                        nc.vector.tensor_copy(fbuf[64:128, 4:100],
                                              p2v[:, 32:128, 1])
                    else:
                        base = 4 + (grp * 4 - 1) * 32
                        nc.vector.tensor_copy(fbuf[0:64, base:base + n * 32],
                                              p2v[:, 0:n * 32, 0])
                        nc.vector.tensor_copy(fbuf[64:128, base:base + n * 32],
                                              p2v[:, 0:n * 32, 1])

            # ================= phase 2: conv ================================
            with tc.tile_pool(name="pscv2", bufs=1, space="PSUM") as pscv2:
                psc = [pscv2.tile([128, 512], F32, tag=f"psc{i}",
                                  name=f"psc{i}") for i in range(NBLK)]
                for d in range(4):
                    for blk in range(NBLK):
                        o = 4 + blk * 512 - d
                        nc.tensor.matmul(psc[blk][:], tsb[:, d, :],
                                         fbuf[:, o:o + 512],
                                         start=(d == 0), stop=(d == 3))
                for blk in range(NBLK):
                    nc.vector.tensor_copy(frlin[:, blk * 512:(blk + 1) * 512],
                                          psc[blk][:])

            frl3 = frlin.rearrange("p (b k) -> p b k", b=BS)

            # ======= L1 / L3: acts-stationary 12288->512 + relu + transpose =
            def big_layer(src_blk, d_w, bias_row, out_sb, outT, dma_eng):
                with tc.tile_pool(name="wp", bufs=10) as wp, \
                     tc.tile_pool(name="psm", bufs=1, space="PSUM") as psm, \
                     tc.tile_pool(name="pst", bufs=1, space="PSUM") as pst:
                    acc = psm.tile([32, 512], F32, tag="acc")
                    for K4 in range(24):
                        wt = wp.tile([128, 4, 512], F16, tag="wt")
                        dma_eng.dma_start(wt[:], d_w[K4])
                        for j in range(4):
                            nc.tensor.matmul(acc[:], src_blk(4 * K4 + j),
                                             wt[:, j, :],
                                             start=(K4 == 0 and j == 0),
                                             stop=False)
                    nc.tensor.matmul(acc[:], ones1[:], bias_row[:],
                                     start=False, stop=True)
                    nc.scalar.activation(out_sb[:], acc[:], AF.Relu)
                    pt = pst.tile([128, 128], F16, tag="pt")
                    for fb in range(4):
                        nc.tensor.transpose(pt[:, fb * 32:(fb + 1) * 32],
                                            out_sb[:, fb * 128:(fb + 1) * 128],
                                            idn[:])
                    nc.vector.tensor_copy(
                        outT[:], pt.rearrange("p (f b) -> p f b", f=4))

            big_layer(lambda K: frl3[:, :, K], d_w1, b1s, h_sb, hT, nc.sync)

            # ======= L2: acts-stationary 512->12288 -> specT ================
            nc.sync.dma_start(bbig[:], d_b2[:])
            with tc.tile_pool(name="wp2", bufs=3) as wp2, \
                 tc.tile_pool(name="sp2", bufs=2) as sp2, \
                 tc.tile_pool(name="ps2m", bufs=2, space="PSUM") as ps2m, \
                 tc.tile_pool(name="pst2", bufs=2, space="PSUM") as pst2:
                for mc2 in range(12):
                    wt = wp2.tile([128, 4, 1024], F16, tag="w2")
                    nc.sync.dma_start(wt[:], d_w2[mc2])
                    for half in range(2):
                        mc = 2 * mc2 + half
                        acc = ps2m.tile([32, 512], F32, tag="acc2")
                        for fb in range(4):
                            nc.tensor.matmul(
                                acc[:], hT[:, fb, :],
                                wt[:, fb, half * 512:(half + 1) * 512],
                                start=(fb == 0), stop=False)
                        nc.tensor.matmul(acc[:], ones1[:],
                                         bbig[0:1, mc * 512:(mc + 1) * 512],
                                         start=False, stop=True)
                        sb = sp2.tile([32, 512], F16, tag="sb2")
                        if half == 0:
                            nc.scalar.copy(sb[:], acc[:])
                        else:
                            nc.vector.tensor_copy(sb[:], acc[:])
                        pt = pst2.tile([128, 128], F16, tag="pt2")
                        for fb in range(4):
                            nc.tensor.transpose(pt[:, fb * 32:(fb + 1) * 32],
                                                sb[:, fb * 128:(fb + 1) * 128],
                                                idn[:])
                        nc.vector.tensor_copy(
                            specT[:, mc * 4:(mc + 1) * 4, :],
                            pt.rearrange("p (f b) -> p f b", f=4))

            # ======= L3 =====================================================
            big_layer(lambda K: specT[:, K, :], d_w3, b3s, h1_sb, h1T,
                      nc.gpsimd)

            # ======= L4: weights-stationary 512->512 + relu =================
            with tc.tile_pool(name="wp4", bufs=1) as wp4, \
                 tc.tile_pool(name="ps4m", bufs=2, space="PSUM") as ps4m:
                w4 = wp4.tile([128, 4, 512], F16, tag="w4")
                nc.gpsimd.dma_start(w4[:], d_w4.rearrange("a p k -> p a k"))
                for f2b in range(4):
                    acc = ps4m.tile([128, 32], F32, tag="acc4")
                    for fb in range(4):
                        nc.tensor.matmul(acc[:],
                                         w4[:, f2b, fb * 128:(fb + 1) * 128],
                                         h1T[:, fb, :],
                                         start=(fb == 0), stop=(fb == 3))
                    nc.scalar.activation(h2T[:, f2b, :], acc[:], AF.Relu,
                                         bias=b4s[:, f2b:f2b + 1])

            # ======= L5: acts-stationary 512->12288 -> procT (64-blocks) ====
            nc.sync.dma_start(bbig[:], d_b5[:])
            with tc.tile_pool(name="wp5", bufs=3) as wp5, \
                 tc.tile_pool(name="sp5", bufs=2) as sp5, \
                 tc.tile_pool(name="ps5m", bufs=2, space="PSUM") as ps5m, \
                 tc.tile_pool(name="pst5", bufs=2, space="PSUM") as pst5:
                for mc2 in range(12):
                    wt = wp5.tile([128, 4, 1024], F16, tag="w5")
                    nc.gpsimd.dma_start(wt[:], d_w5[mc2])
                    for half in range(2):
                        mc = 2 * mc2 + half
                        acc = ps5m.tile([32, 512], F32, tag="acc5")
                        for fb in range(4):
                            nc.tensor.matmul(
                                acc[:], h2T[:, fb, :],
                                wt[:, fb, half * 512:(half + 1) * 512],
                                start=(fb == 0), stop=False)
                        nc.tensor.matmul(acc[:], ones1[:],
                                         bbig[0:1, mc * 512:(mc + 1) * 512],
                                         start=False, stop=True)
                        sb = sp5.tile([32, 512], F16, tag="sb5")
                        if half == 0:
                            nc.scalar.copy(sb[:], acc[:])
                        else:
                            nc.vector.tensor_copy(sb[:], acc[:])
                        pt = pst5.tile([64, 256], F16, tag="pt5")
                        for t in range(8):
                            nc.tensor.transpose(pt[:, t * 32:(t + 1) * 32],
                                                sb[:, t * 64:(t + 1) * 64],
                                                idn[:])
                        nc.vector.tensor_copy(
                            procT[:, mc * 8:(mc + 1) * 8, :],
                            pt.rearrange("p (t b) -> p t b", t=8))

            # ================= ifft2 (per-image m1, 8-wide m2) ==============
            with tc.tile_pool(name="opool", bufs=1) as opool, \
                 tc.tile_pool(name="gpi", bufs=2) as gpi, \
                 tc.tile_pool(name="ps1i", bufs=2, space="PSUM") as ps1i, \
                 tc.tile_pool(name="ps2i", bufs=2, space="PSUM") as ps2i:
                oall = opool.tile([64, NIMG * 64], F32, tag="oall")
                oal3 = oall.rearrange("u (b c v) -> u b c v", b=BS, c=C)
                for c in range(C):
                    for bg in range(BS // 4):
                        psA = ps1i.tile([64, 512], F32, tag="psAi")
                        for t in range(4):
                            b = bg * 4 + t
                            nc.tensor.matmul(psA[:, t * 128:(t + 1) * 128],
                                             procT[:, c * 64:(c + 1) * 64, b],
                                             cswi[:], start=True, stop=True)
                        g1w = gpi.tile([64, 4, 128], F16, tag="g1i")
                        nc.scalar.copy(g1w.rearrange("p a k -> p (a k)"), psA[:])
                        ps2 = ps2i.tile([64, 256], F32, tag="p2i")
                        nc.tensor.matmul(ps2[:], cmi[:], g1w[:, :, 0:64],
                                         start=True, stop=False)
                        nc.tensor.matmul(ps2[:], smi[:], g1w[:, :, 64:128],
                                         start=False, stop=True)
                        nc.scalar.copy(
                            oal3[:, bg * 4:(bg + 1) * 4, c, :],
                            ps2.rearrange("u (b v) -> u b v", b=4))
                for b in range(0, BS, 2):
                    nc.sync.dma_start(
                        d_out[b:b + 2].rearrange("b c u v -> u b c v"),
                        oall[:, b * 192:(b + 2) * 192].rearrange(
                            "u (b c v) -> u b c v", b=2, c=C))

    nc.compile()
    return nc


def _get_nc():
    global _NC_CACHE
    if _NC_CACHE is None:
        _NC_CACHE = _build_nc()
    return _NC_CACHE


def _make_in_maps(x, Ws1, bs1, Ws2, bs2, Wn1, bn1, Wn2, bn2, Wn3, bn3):
    shared = dict(_host_constants())
    shared.update(_prep_weights(Ws1, bs1, Ws2, bs2, Wn1, bn1, Wn2, bn2,
                                Wn3, bn3))
    in_maps = []
    for g in range(NCORE):
        if g == 0:
            halo = np.zeros((1, 64, 64), np.float32)
        else:
            halo = x[g * BS - 1, 2][None]
        ximgs = np.concatenate(
            [halo, x[g * BS:(g + 1) * BS].reshape(NIMG, 64, 64),
             np.zeros((1, 64, 64), np.float32)]).astype(np.float16)
        in_maps.append({"ximgs": np.ascontiguousarray(ximgs), **shared})
    return in_maps


def kernel(**inputs):
    x = np.ascontiguousarray(inputs["x"], dtype=np.float32)
    nc = _get_nc()
    in_maps = _make_in_maps(
        x, inputs["Ws1"], inputs["bs1"], inputs["Ws2"], inputs["bs2"],
        inputs["Wn1"], inputs["bn1"], inputs["Wn2"], inputs["bn2"],
        inputs["Wn3"], inputs["bn3"])
    res = run_bass_kernel_spmd(nc, in_maps, list(range(NCORE)))
    out = np.empty((B, C, H, W), np.float32)
    for g in range(NCORE):
        out[g * BS:(g + 1) * BS] = res.results[g]["out"]
    return out


# revision 30
# speedup vs baseline: 576.1081x; 1.1076x over previous
"""Trainium2 Bass kernel for nn_EnhancedFractionalPINO.

Pipeline (per core, batch-parallel over 8 NeuronCores, 32 batches/core):
  1. f = Re(fft2(x)) per 64x64 image via cosine/sine DFT matmuls:
     m1: per image, lhsT = image, rhs = [C | S] -> [x^T C | x^T S];
     m2: per 8-image group, two const-stationary matmuls with strided rhs
     -> A^T = C x^T C - S x^T S for all 8 images in one psum tile.
  2. GL fractional derivative = truncated causal conv (KTAPS taps) over the
     globally-flattened signal, as Toeplitz-block matmuls (halo image passed
     from the previous core's batch range; zeros for core 0). The h^-alpha
     scale is folded into Ws1 so everything stays in fp16 range.
  3. spectral_operator + neural_operator MLPs as fp16 PE matmuls with a
     positive rescaling chain (LAM_*) keeping activations in fp16 range;
     activations-stationary, PE transposes between layers.
  4. out = Re(ifft2(proc)) via the same DFT-matmul machinery (scales folded
     into the second-stage constants).

Weights are replicated across cores; activations stay SBUF-resident.
"""

import numpy as np

import concourse.bass as bass
import concourse.mybir as mybir
import concourse.tile as tile
from concourse import bacc
from concourse.bass_utils import run_bass_kernel_spmd

F32 = mybir.dt.float32
F16 = mybir.dt.float16
AF = mybir.ActivationFunctionType

B, C, H, W = 256, 3, 64, 64
MODES = C * H * W              # 12288
ALPHA = 0.5
NTOT = B * MODES               # 3145728 flattened samples
NCORE = 8
BS = B // NCORE                # 32 batches per core
NIMG = BS * C                  # 96 images per core
NSLOT = NIMG + 2               # halo + 96 images + zero pad
KTAPS = 512                    # truncated GL taps (4 chunks of 128)
NCH = BS * MODES // 128        # 3072 output chunks per core
NBLK = NCH // 512              # 6 conv blocks of 512 chunks

# fp16 activation rescaling chain (see mirror3 validation)
LAM_H, LAM_S, LAM_1, LAM_2, LAM_P = 16.0, 8.0, 4.0, 4.0, 4.0


# ---------------------------------------------------------------- host consts
def _host_constants():
    jk = np.outer(np.arange(64), np.arange(64)).astype(np.float64)
    Cm = np.cos(2 * np.pi * jk / 64)
    Sm = np.sin(2 * np.pi * jk / 64)

    j = np.arange(1, KTAPS, dtype=np.float64)
    w = np.concatenate([[1.0], np.cumprod((j - 1.0 - ALPHA) / j)])

    # Tst[d][t, tau] = w[128*d + tau - t]  (lhsT layout of the Toeplitz blocks)
    idx = 128 * np.arange(4)[:, None, None] \
        + np.arange(128)[None, None, :] - np.arange(128)[None, :, None]
    Tst = np.where((idx >= 0) & (idx < KTAPS), w[np.clip(idx, 0, KTAPS - 1)], 0.0)

    f16 = lambda a: np.ascontiguousarray(a, dtype=np.float16)
    return {
        "cswi": f16(np.concatenate([Cm, Sm], axis=1)),     # [64, 128]
        "cmf": f16(Cm),                                    # [64, 64]
        "msf": f16(-Sm),
        "cmi": f16(Cm * (LAM_P / 4096.0)),
        "smi": f16(-Sm * (LAM_P / 4096.0)),
        "tst": f16(Tst),
        "idn32": f16(np.eye(32)),
        "ones1": f16(np.ones((1, 32))),
    }


def _prep_weights(Ws1, bs1, Ws2, bs2, Wn1, bn1, Wn2, bn2, Wn3, bn3):
    s = float(np.float64(1.0 / (NTOT - 1)) ** (-ALPHA))
    f16 = lambda a: np.ascontiguousarray(a, dtype=np.float16)
    W1 = (Ws1.astype(np.float64) * (s / LAM_H)).astype(np.float32)
    W2 = Ws2 * np.float32(LAM_H / LAM_S)
    W3 = Wn1 * np.float32(LAM_S / LAM_1)
    W4 = Wn2 * np.float32(LAM_1 / LAM_2)
    W5 = Wn3 * np.float32(LAM_2 / LAM_P)
    return {
        "w1t": f16(W1.reshape(24, 4, 128, 512).transpose(0, 2, 1, 3)),
        "w2r": f16(W2.reshape(4, 128, 12, 1024).transpose(2, 1, 0, 3)),
        "w3t": f16(W3.reshape(24, 4, 128, 512).transpose(0, 2, 1, 3)),
        "w4t": f16(W4.reshape(4, 128, 4, 128).transpose(2, 1, 0, 3)
                   .reshape(4, 128, 512)),
        "w5r": f16(W5.reshape(4, 128, 12, 1024).transpose(2, 1, 0, 3)),
        "b1r": f16((bs1 / LAM_H).reshape(1, 512)),
        "b2r": f16((bs2 / LAM_S).reshape(1, MODES)),
        "b3r": f16((bn1 / LAM_1).reshape(1, 512)),
        "b4t": np.ascontiguousarray((bn2 / LAM_2).reshape(4, 128).T,
                                    dtype=np.float32),     # [128, 4]
        "b5r": f16((bn3 / LAM_P).reshape(1, MODES)),
    }


# ---------------------------------------------------------------- bass module
_NC_CACHE = None


def _build_nc():
    nc = bacc.Bacc("TRN2", target_bir_lowering=False, debug=False,
                   num_devices=NCORE)

    def din(name, shape, dt=F16):
        return nc.dram_tensor(name, shape, dt, kind="ExternalInput")

    d_x = din("ximgs", (NSLOT, 64, 64))
    d_cswi = din("cswi", (64, 128))
    d_cmf = din("cmf", (64, 64))
    d_msf = din("msf", (64, 64))
    d_cmi = din("cmi", (64, 64))
    d_smi = din("smi", (64, 64))
    d_tst = din("tst", (4, 128, 128))
    d_idn = din("idn32", (32, 32))
    d_ones = din("ones1", (1, 32))
    d_w1 = din("w1t", (24, 128, 4, 512))
    d_w2 = din("w2r", (12, 128, 4, 1024))
    d_w3 = din("w3t", (24, 128, 4, 512))
    d_w4 = din("w4t", (4, 128, 512))
    d_w5 = din("w5r", (12, 128, 4, 1024))
    d_b1 = din("b1r", (1, 512))
    d_b2 = din("b2r", (1, MODES))
    d_b3 = din("b3r", (1, 512))
    d_b4 = nc.dram_tensor("b4t", (128, 4), F32, kind="ExternalInput")
    d_b5 = din("b5r", (1, MODES))
    d_out = nc.dram_tensor("out", (BS, C, 64, 64), F32, kind="ExternalOutput")

    with tile.TileContext(nc) as tc:
        with tc.tile_pool(name="cpool", bufs=1) as cpool, \
             tc.tile_pool(name="bigpool", bufs=1) as bigpool:
            # ---- constants into SBUF
            cswi = cpool.tile([64, 128], F16, tag="cswi")
            cmf = cpool.tile([64, 64], F16, tag="cmf")
            msf = cpool.tile([64, 64], F16, tag="msf")
            cmi = cpool.tile([64, 64], F16, tag="cmi")
            smi = cpool.tile([64, 64], F16, tag="smi")
            tsb = cpool.tile([128, 4, 128], F16, tag="tsb")
            idn = cpool.tile([32, 32], F16, tag="idn")
            ones1 = cpool.tile([1, 32], F16, tag="ones1")
            b1s = cpool.tile([1, 512], F16, tag="b1s")
            b3s = cpool.tile([1, 512], F16, tag="b3s")
            b4s = cpool.tile([128, 4], F32, tag="b4s")
            bbig = cpool.tile([1, MODES], F16, tag="bbig")  # b2 then b5
            for t, d in ((cswi, d_cswi), (cmf, d_cmf), (msf, d_msf),
                         (cmi, d_cmi), (smi, d_smi), (idn, d_idn),
                         (ones1, d_ones), (b1s, d_b1), (b3s, d_b3),
                         (b4s, d_b4)):
                nc.sync.dma_start(t[:], d[:])
            nc.sync.dma_start(tsb[:], d_tst.rearrange("d p k -> p d k"))

            # ---- persistent activation tiles
            fbuf = bigpool.tile([128, 4 + NCH + 64], F16, tag="fbuf")
            frlin = bigpool.tile([128, NCH], F16, tag="frlin")
            specT = bigpool.tile([128, 96, BS], F16, tag="specT")
            procTs = [bigpool.tile([64, 64, BS], F16, tag=f"procT{i}",
                                   name=f"procT{i}") for i in range(C)]
            hT = bigpool.tile([128, 4, BS], F16, tag="hT")
            h1T = bigpool.tile([128, 4, BS], F16, tag="h1T")
            h2T = bigpool.tile([128, 4, BS], F16, tag="h2T")
            h_sb = bigpool.tile([32, 512], F16, tag="h_sb")
            h1_sb = bigpool.tile([32, 512], F16, tag="h1_sb")

            # ========== phase 1: fft2 (per-image m1, 8-wide m2) =============
            with tc.tile_pool(name="xpool", bufs=1) as xpool, \
                 tc.tile_pool(name="gpool", bufs=4) as gpool, \
                 tc.tile_pool(name="ps1p", bufs=4, space="PSUM") as ps1p, \
                 tc.tile_pool(name="ps2p", bufs=3, space="PSUM") as ps2p:
                xall = xpool.tile([64, NSLOT, 64], F16, tag="xall")
                for ch in range(4):
                    q0 = (NSLOT * ch) // 4
                    q1 = (NSLOT * (ch + 1)) // 4
                    nc.sync.dma_start(
                        xall[:, q0:q1, :],
                        d_x[q0:q1].rearrange("q p k -> p q k"))
                for grp in range(25):
                    n = 4 if grp < 24 else 2
                    psA = ps1p.tile([64, 512], F32, tag="psA")
                    for t in range(n):
                        i = grp * 4 + t
                        nc.tensor.matmul(psA[:, t * 128:(t + 1) * 128],
                                         xall[:, i, :], cswi[:],
                                         start=True, stop=True)
                    g1w = gpool.tile([64, 4, 128], F16, tag="g1w")
                    nc.scalar.copy(g1w[:, 0:n, :].rearrange("p a k -> p (a k)"),
                                   psA[:, 0:n * 128])
                    ps2 = ps2p.tile([64, 256], F32, tag="ps2")
                    nc.tensor.matmul(ps2[:, 0:n * 64], cmf[:],
                                     g1w[:, 0:n, 0:64], start=True, stop=False)
                    nc.tensor.matmul(ps2[:, 0:n * 64], msf[:],
                                     g1w[:, 0:n, 64:128], start=False, stop=True)
                    p2v = ps2.rearrange("p (k two) -> p k two", two=2)
                    if grp == 0:
                        # halo image: last 4 chunk-cols; imgs 1..3 -> cols 4:100
                        nc.vector.tensor_copy(fbuf[0:64, 0:4], p2v[:, 28:32, 0])
                        nc.vector.tensor_copy(fbuf[64:128, 0:4], p2v[:, 28:32, 1])
                        nc.vector.tensor_copy(fbuf[0:64, 4:100], p2v[:, 32:128, 0])
                        nc.vector.tensor_copy(fbuf[64:128, 4:100],
                                              p2v[:, 32:128, 1])
                    else:
                        base = 4 + (grp * 4 - 1) * 32
                        nc.vector.tensor_copy(fbuf[0:64, base:base + n * 32],
                                              p2v[:, 0:n * 32, 0])
                        nc.vector.tensor_copy(fbuf[64:128, base:base + n * 32],
                                              p2v[:, 0:n * 32, 1])

            # ================= phase 2: conv ================================
            with tc.tile_pool(name="pscv2", bufs=1, space="PSUM") as pscv2:
                psc = [pscv2.tile([128, 512], F32, tag=f"psc{i}",
                                  name=f"psc{i}") for i in range(NBLK)]
                for d in range(4):
                    for blk in range(NBLK):
                        o = 4 + blk * 512 - d
                        nc.tensor.matmul(psc[blk][:], tsb[:, d, :],
                                         fbuf[:, o:o + 512],
                                         start=(d == 0), stop=(d == 3))
                for blk in range(NBLK):
                    nc.vector.tensor_copy(frlin[:, blk * 512:(blk + 1) * 512],
                                          psc[blk][:])

            frl3 = frlin.rearrange("p (b k) -> p b k", b=BS)

            # ======= L1 / L3: acts-stationary 12288->512 + relu + transpose =
            def big_layer(src_blk, d_w, bias_row, out_sb, outT, dma_eng):
                with tc.tile_pool(name="wp", bufs=10) as wp, \
                     tc.tile_pool(name="psm", bufs=1, space="PSUM") as psm, \
                     tc.tile_pool(name="pst", bufs=1, space="PSUM") as pst:
                    acc = psm.tile([32, 512], F32, tag="acc")
                    for K4 in range(24):
                        wt = wp.tile([128, 4, 512], F16, tag="wt")
                        dma_eng.dma_start(wt[:], d_w[K4])
                        for j in range(4):
                            nc.tensor.matmul(acc[:], src_blk(4 * K4 + j),
                                             wt[:, j, :],
                                             start=(K4 == 0 and j == 0),
                                             stop=False)
                    nc.tensor.matmul(acc[:], ones1[:], bias_row[:],
                                     start=False, stop=True)
                    nc.scalar.activation(out_sb[:], acc[:], AF.Relu)
                    pt = pst.tile([128, 128], F16, tag="pt")
                    for fb in range(4):
                        nc.tensor.transpose(pt[:, fb * 32:(fb + 1) * 32],
                                            out_sb[:, fb * 128:(fb + 1) * 128],
                                            idn[:])
                    nc.vector.tensor_copy(
                        outT[:], pt.rearrange("p (f b) -> p f b", f=4))

            big_layer(lambda K: frl3[:, :, K], d_w1, b1s, h_sb, hT, nc.sync)

            # ======= L2 + L3, emission-interleaved ==========================
            # L3's k-block K only needs L2's chunk K//4, and PSUM accumulation
            # is order-independent, so L3's matmuls ride along the L2 loop.
            nc.sync.dma_start(bbig[:], d_b2[:])
            with tc.tile_pool(name="wp2", bufs=3) as wp2, \
                 tc.tile_pool(name="wp3", bufs=3) as wp3, \
                 tc.tile_pool(name="sp2", bufs=3) as sp2, \
                 tc.tile_pool(name="ps2m", bufs=3, space="PSUM") as ps2m, \
                 tc.tile_pool(name="pst2", bufs=3, space="PSUM") as pst2, \
                 tc.tile_pool(name="psm3", bufs=1, space="PSUM") as psm3:
                acc3 = psm3.tile([32, 512], F32, tag="acc3")
                for mc2 in range(12):
                    wt = wp2.tile([128, 4, 1024], F16, tag="w2")
                    nc.sync.dma_start(wt[:], d_w2[mc2])
                    for half in range(2):
                        mc = 2 * mc2 + half
                        acc = ps2m.tile([32, 512], F32, tag="acc2")
                        for fb in range(4):
                            nc.tensor.matmul(
                                acc[:], hT[:, fb, :],
                                wt[:, fb, half * 512:(half + 1) * 512],
                                start=(fb == 0), stop=False)
                        nc.tensor.matmul(acc[:], ones1[:],
                                         bbig[0:1, mc * 512:(mc + 1) * 512],
                                         start=False, stop=True)
                        sb = sp2.tile([32, 512], F16, tag="sb2")
                        if half == 0:
                            nc.scalar.copy(sb[:], acc[:])
                        else:
                            nc.vector.tensor_copy(sb[:], acc[:])
                        pt = pst2.tile([128, 128], F16, tag="pt2")
                        for fb in range(4):
                            nc.tensor.transpose(pt[:, fb * 32:(fb + 1) * 32],
                                                sb[:, fb * 128:(fb + 1) * 128],
                                                idn[:])
                        nc.vector.tensor_copy(
                            specT[:, mc * 4:(mc + 1) * 4, :],
                            pt.rearrange("p (f b) -> p f b", f=4))
                    # L3 portion: k-blocks for the two chunks just produced
                    wt3 = wp3.tile([128, 4, 512], F16, tag="wt3")
                    nc.scalar.dma_start(wt3[:], d_w3[2 * mc2])
                    wt3b = wp3.tile([128, 4, 512], F16, tag="wt3b")
                    nc.scalar.dma_start(wt3b[:], d_w3[2 * mc2 + 1])
                    for K4, w3t in ((2 * mc2, wt3), (2 * mc2 + 1, wt3b)):
                        for j in range(4):
                            nc.tensor.matmul(acc3[:],
                                             specT[:, 4 * K4 + j, :],
                                             w3t[:, j, :],
                                             start=(mc2 == 0 and K4 == 0
                                                    and j == 0),
                                             stop=False)
                nc.tensor.matmul(acc3[:], ones1[:], b3s[:],
                                 start=False, stop=True)
                nc.scalar.activation(h1_sb[:], acc3[:], AF.Relu)
                with tc.tile_pool(name="pst3", bufs=1, space="PSUM") as pst3:
                    pt = pst3.tile([128, 128], F16, tag="pt3")
                    for fb in range(4):
                        nc.tensor.transpose(pt[:, fb * 32:(fb + 1) * 32],
                                            h1_sb[:, fb * 128:(fb + 1) * 128],
                                            idn[:])
                    nc.vector.tensor_copy(
                        h1T[:], pt.rearrange("p (f b) -> p f b", f=4))

            # ======= L4: weights-stationary 512->512 + relu =================
            with tc.tile_pool(name="wp4", bufs=1) as wp4, \
                 tc.tile_pool(name="ps4m", bufs=2, space="PSUM") as ps4m:
                w4 = wp4.tile([128, 4, 512], F16, tag="w4")
                nc.gpsimd.dma_start(w4[:], d_w4.rearrange("a p k -> p a k"))
                for f2b in range(4):
                    acc = ps4m.tile([128, 32], F32, tag="acc4")
                    for fb in range(4):
                        nc.tensor.matmul(acc[:],
                                         w4[:, f2b, fb * 128:(fb + 1) * 128],
                                         h1T[:, fb, :],
                                         start=(fb == 0), stop=(fb == 3))
                    nc.scalar.activation(h2T[:, f2b, :], acc[:], AF.Relu,
                                         bias=b4s[:, f2b:f2b + 1])

            # ======= L5 + ifft2, emission-interleaved by channel ============
            nc.sync.dma_start(bbig[:], d_b5[:])
            with tc.tile_pool(name="wp5", bufs=3) as wp5, \
                 tc.tile_pool(name="sp5", bufs=3) as sp5, \
                 tc.tile_pool(name="opool", bufs=1) as opool, \
                 tc.tile_pool(name="gpi", bufs=2) as gpi, \
                 tc.tile_pool(name="ps5m", bufs=2, space="PSUM") as ps5m, \
                 tc.tile_pool(name="pst5", bufs=2, space="PSUM") as pst5, \
                 tc.tile_pool(name="ps1i", bufs=2, space="PSUM") as ps1i, \
                 tc.tile_pool(name="ps2i", bufs=2, space="PSUM") as ps2i:
                oall = opool.tile([64, NIMG * 64], F32, tag="oall")
                oal3 = oall.rearrange("u (b c v) -> u b c v", b=BS, c=C)

                def ifft2_channel(c):
                    for bg in range(BS // 4):
                        psA = ps1i.tile([64, 512], F32, tag="psAi",
                                        name="psAi")
                        for t in range(4):
                            b = bg * 4 + t
                            nc.tensor.matmul(psA[:, t * 128:(t + 1) * 128],
                                             procTs[c][:, :, b],
                                             cswi[:], start=True, stop=True)
                        g1w = gpi.tile([64, 4, 128], F16, tag="g1i",
                                       name="g1i")
                        nc.scalar.copy(g1w.rearrange("p a k -> p (a k)"),
                                       psA[:])
                        ps2 = ps2i.tile([64, 256], F32, tag="p2i", name="p2i")
                        nc.tensor.matmul(ps2[:], cmi[:], g1w[:, :, 0:64],
                                         start=True, stop=False)
                        nc.tensor.matmul(ps2[:], smi[:], g1w[:, :, 64:128],
                                         start=False, stop=True)
                        nc.scalar.copy(
                            oal3[:, bg * 4:(bg + 1) * 4, c, :],
                            ps2.rearrange("u (b v) -> u b v", b=4))

                for mc2 in range(12):
                    wt = wp5.tile([128, 4, 1024], F16, tag="w5")
                    nc.gpsimd.dma_start(wt[:], d_w5[mc2])
                    for half in range(2):
                        mc = 2 * mc2 + half
                        acc = ps5m.tile([32, 512], F32, tag="acc5")
                        for fb in range(4):
                            nc.tensor.matmul(
                                acc[:], h2T[:, fb, :],
                                wt[:, fb, half * 512:(half + 1) * 512],
                                start=(fb == 0), stop=False)
                        nc.tensor.matmul(acc[:], ones1[:],
                                         bbig[0:1, mc * 512:(mc + 1) * 512],
                                         start=False, stop=True)
                        sb = sp5.tile([32, 512], F16, tag="sb5")
                        if half == 0:
                            nc.scalar.copy(sb[:], acc[:])
                        else:
                            nc.vector.tensor_copy(sb[:], acc[:])
                        pt = pst5.tile([64, 256], F16, tag="pt5")
                        for t in range(8):
                            nc.tensor.transpose(pt[:, t * 32:(t + 1) * 32],
                                                sb[:, t * 64:(t + 1) * 64],
                                                idn[:])
                        nc.vector.tensor_copy(
                            procTs[mc // 8][:, (mc % 8) * 8:(mc % 8 + 1) * 8, :],
                            pt.rearrange("p (t b) -> p t b", t=8))
                    if mc2 in (3, 7, 11):
                        ifft2_channel(mc2 // 4)
                for b in range(0, BS, 2):
                    nc.sync.dma_start(
                        d_out[b:b + 2].rearrange("b c u v -> u b c v"),
                        oall[:, b * 192:(b + 2) * 192].rearrange(
                            "u (b c v) -> u b c v", b=2, c=C))

    nc.compile()
    return nc


def _get_nc():
    global _NC_CACHE
    if _NC_CACHE is None:
        _NC_CACHE = _build_nc()
    return _NC_CACHE


def _make_in_maps(x, Ws1, bs1, Ws2, bs2, Wn1, bn1, Wn2, bn2, Wn3, bn3):
    shared = dict(_host_constants())
    shared.update(_prep_weights(Ws1, bs1, Ws2, bs2, Wn1, bn1, Wn2, bn2,
                                Wn3, bn3))
    in_maps = []
    for g in range(NCORE):
        if g == 0:
            halo = np.zeros((1, 64, 64), np.float32)
        else:
            halo = x[g * BS - 1, 2][None]
        ximgs = np.concatenate(
            [halo, x[g * BS:(g + 1) * BS].reshape(NIMG, 64, 64),
             np.zeros((1, 64, 64), np.float32)]).astype(np.float16)
        in_maps.append({"ximgs": np.ascontiguousarray(ximgs), **shared})
    return in_maps


def kernel(**inputs):
    x = np.ascontiguousarray(inputs["x"], dtype=np.float32)
    nc = _get_nc()
    in_maps = _make_in_maps(
        x, inputs["Ws1"], inputs["bs1"], inputs["Ws2"], inputs["bs2"],
        inputs["Wn1"], inputs["bn1"], inputs["Wn2"], inputs["bn2"],
        inputs["Wn3"], inputs["bn3"])
    res = run_bass_kernel_spmd(nc, in_maps, list(range(NCORE)))
    out = np.empty((B, C, H, W), np.float32)
    for g in range(NCORE):
        out[g * BS:(g + 1) * BS] = res.results[g]["out"]
    return out


# revision 32
# speedup vs baseline: 578.4178x; 1.0040x over previous
"""Trainium2 Bass kernel for nn_EnhancedFractionalPINO.

Pipeline (per core, batch-parallel over 8 NeuronCores, 32 batches/core):
  1. f = Re(fft2(x)) per 64x64 image via cosine/sine DFT matmuls:
     m1: per image, lhsT = image, rhs = [C | S] -> [x^T C | x^T S];
     m2: per 8-image group, two const-stationary matmuls with strided rhs
     -> A^T = C x^T C - S x^T S for all 8 images in one psum tile.
  2. GL fractional derivative = truncated causal conv (KTAPS taps) over the
     globally-flattened signal, as Toeplitz-block matmuls (halo image passed
     from the previous core's batch range; zeros for core 0). The h^-alpha
     scale is folded into Ws1 so everything stays in fp16 range.
  3. spectral_operator + neural_operator MLPs as fp16 PE matmuls with a
     positive rescaling chain (LAM_*) keeping activations in fp16 range;
     activations-stationary, PE transposes between layers.
  4. out = Re(ifft2(proc)) via the same DFT-matmul machinery (scales folded
     into the second-stage constants).

Weights are replicated across cores; activations stay SBUF-resident.
"""

import numpy as np

import concourse.bass as bass
import concourse.mybir as mybir
import concourse.tile as tile
from concourse import bacc
from concourse.bass_utils import run_bass_kernel_spmd

F32 = mybir.dt.float32
F16 = mybir.dt.float16
AF = mybir.ActivationFunctionType

B, C, H, W = 256, 3, 64, 64
MODES = C * H * W              # 12288
ALPHA = 0.5
NTOT = B * MODES               # 3145728 flattened samples
NCORE = 8
BS = B // NCORE                # 32 batches per core
NIMG = BS * C                  # 96 images per core
NSLOT = NIMG + 2               # halo + 96 images + zero pad
KTAPS = 512                    # truncated GL taps (4 chunks of 128)
NCH = BS * MODES // 128        # 3072 output chunks per core
NBLK = NCH // 512              # 6 conv blocks of 512 chunks

# fp16 activation rescaling chain (see mirror3 validation)
LAM_H, LAM_S, LAM_1, LAM_2, LAM_P = 16.0, 8.0, 4.0, 4.0, 4.0


# ---------------------------------------------------------------- host consts
def _host_constants():
    jk = np.outer(np.arange(64), np.arange(64)).astype(np.float64)
    Cm = np.cos(2 * np.pi * jk / 64)
    Sm = np.sin(2 * np.pi * jk / 64)

    j = np.arange(1, KTAPS, dtype=np.float64)
    w = np.concatenate([[1.0], np.cumprod((j - 1.0 - ALPHA) / j)])

    # Tst[d][t, tau] = w[128*d + tau - t]  (lhsT layout of the Toeplitz blocks)
    idx = 128 * np.arange(4)[:, None, None] \
        + np.arange(128)[None, None, :] - np.arange(128)[None, :, None]
    Tst = np.where((idx >= 0) & (idx < KTAPS), w[np.clip(idx, 0, KTAPS - 1)], 0.0)

    f16 = lambda a: np.ascontiguousarray(a, dtype=np.float16)
    return {
        "cswi": f16(np.concatenate([Cm, Sm], axis=1)),     # [64, 128]
        "cmf": f16(Cm),                                    # [64, 64]
        "msf": f16(-Sm),
        "cmi": f16(Cm * (LAM_P / 4096.0)),
        "smi": f16(-Sm * (LAM_P / 4096.0)),
        "tst": f16(Tst),
        "idn32": f16(np.eye(32)),
        "ones1": f16(np.ones((1, 32))),
    }


def _prep_weights(Ws1, bs1, Ws2, bs2, Wn1, bn1, Wn2, bn2, Wn3, bn3):
    s = float(np.float64(1.0 / (NTOT - 1)) ** (-ALPHA))
    f16 = lambda a: np.ascontiguousarray(a, dtype=np.float16)
    W1 = (Ws1.astype(np.float64) * (s / LAM_H)).astype(np.float32)
    W2 = Ws2 * np.float32(LAM_H / LAM_S)
    W3 = Wn1 * np.float32(LAM_S / LAM_1)
    W4 = Wn2 * np.float32(LAM_1 / LAM_2)
    W5 = Wn3 * np.float32(LAM_2 / LAM_P)
    return {
        "w1t": f16(W1.reshape(24, 4, 128, 512).transpose(0, 2, 1, 3)),
        "w2r": f16(W2.reshape(4, 128, 12, 1024).transpose(2, 1, 0, 3)),
        "w3t": f16(W3.reshape(24, 4, 128, 512).transpose(0, 2, 1, 3)),
        "w4t": f16(W4.reshape(4, 128, 4, 128).transpose(2, 1, 0, 3)
                   .reshape(4, 128, 512)),
        "w5r": f16(W5.reshape(4, 128, 12, 1024).transpose(2, 1, 0, 3)),
        "b1r": f16((bs1 / LAM_H).reshape(1, 512)),
        "b2r": f16((bs2 / LAM_S).reshape(1, MODES)),
        "b3r": f16((bn1 / LAM_1).reshape(1, 512)),
        "b4t": np.ascontiguousarray((bn2 / LAM_2).reshape(4, 128).T,
                                    dtype=np.float32),     # [128, 4]
        "b5r": f16((bn3 / LAM_P).reshape(1, MODES)),
    }


# ---------------------------------------------------------------- bass module
_NC_CACHE = None


def _build_nc():
    nc = bacc.Bacc("TRN2", target_bir_lowering=False, debug=False,
                   num_devices=NCORE)

    def din(name, shape, dt=F16):
        return nc.dram_tensor(name, shape, dt, kind="ExternalInput")

    d_x = din("ximgs", (NSLOT, 64, 64))
    d_cswi = din("cswi", (64, 128))
    d_cmf = din("cmf", (64, 64))
    d_msf = din("msf", (64, 64))
    d_cmi = din("cmi", (64, 64))
    d_smi = din("smi", (64, 64))
    d_tst = din("tst", (4, 128, 128))
    d_idn = din("idn32", (32, 32))
    d_ones = din("ones1", (1, 32))
    d_w1 = din("w1t", (24, 128, 4, 512))
    d_w2 = din("w2r", (12, 128, 4, 1024))
    d_w3 = din("w3t", (24, 128, 4, 512))
    d_w4 = din("w4t", (4, 128, 512))
    d_w5 = din("w5r", (12, 128, 4, 1024))
    d_b1 = din("b1r", (1, 512))
    d_b2 = din("b2r", (1, MODES))
    d_b3 = din("b3r", (1, 512))
    d_b4 = nc.dram_tensor("b4t", (128, 4), F32, kind="ExternalInput")
    d_b5 = din("b5r", (1, MODES))
    d_out = nc.dram_tensor("out", (BS, C, 64, 64), F32, kind="ExternalOutput")

    with tile.TileContext(nc) as tc:
        with tc.tile_pool(name="cpool", bufs=1) as cpool, \
             tc.tile_pool(name="bigpool", bufs=1) as bigpool:
            # ---- constants into SBUF
            cswi = cpool.tile([64, 128], F16, tag="cswi")
            cmf = cpool.tile([64, 64], F16, tag="cmf")
            msf = cpool.tile([64, 64], F16, tag="msf")
            cmi = cpool.tile([64, 64], F16, tag="cmi")
            smi = cpool.tile([64, 64], F16, tag="smi")
            tsb = cpool.tile([128, 4, 128], F16, tag="tsb")
            idn = cpool.tile([32, 32], F16, tag="idn")
            ones1 = cpool.tile([1, 32], F16, tag="ones1")
            b1s = cpool.tile([1, 512], F16, tag="b1s")
            b3s = cpool.tile([1, 512], F16, tag="b3s")
            b4s = cpool.tile([128, 4], F32, tag="b4s")
            bbig = cpool.tile([1, MODES], F16, tag="bbig")  # b2 then b5
            for t, d in ((cswi, d_cswi), (cmf, d_cmf), (msf, d_msf),
                         (cmi, d_cmi), (smi, d_smi), (idn, d_idn),
                         (ones1, d_ones), (b1s, d_b1), (b3s, d_b3),
                         (b4s, d_b4)):
                nc.sync.dma_start(t[:], d[:])
            nc.sync.dma_start(tsb[:], d_tst.rearrange("d p k -> p d k"))

            # ---- persistent activation tiles
            fbuf = bigpool.tile([128, 4 + NCH + 64], F16, tag="fbuf")
            frlin = bigpool.tile([128, NCH], F16, tag="frlin")
            specT = bigpool.tile([128, 96, BS], F16, tag="specT")
            procTs = [bigpool.tile([64, 64, BS], F16, tag=f"procT{i}",
                                   name=f"procT{i}") for i in range(C)]
            hT = bigpool.tile([128, 4, BS], F16, tag="hT")
            h1T = bigpool.tile([128, 4, BS], F16, tag="h1T")
            h2T = bigpool.tile([128, 4, BS], F16, tag="h2T")
            h_sb = bigpool.tile([32, 512], F16, tag="h_sb")
            h1_sb = bigpool.tile([32, 512], F16, tag="h1_sb")

            # ========== phase 1: fft2 (per-image m1, 8-wide m2) =============
            with tc.tile_pool(name="xpool", bufs=1) as xpool, \
                 tc.tile_pool(name="gpool", bufs=6) as gpool, \
                 tc.tile_pool(name="ps1p", bufs=4, space="PSUM") as ps1p, \
                 tc.tile_pool(name="ps2p", bufs=3, space="PSUM") as ps2p:
                xall = xpool.tile([64, NSLOT, 64], F16, tag="xall")
                for ch in range(4):
                    q0 = (NSLOT * ch) // 4
                    q1 = (NSLOT * (ch + 1)) // 4
                    nc.sync.dma_start(
                        xall[:, q0:q1, :],
                        d_x[q0:q1].rearrange("q p k -> p q k"))
                for grp in range(25):
                    n = 4 if grp < 24 else 2
                    psA = ps1p.tile([64, 512], F32, tag="psA")
                    for t in range(n):
                        i = grp * 4 + t
                        nc.tensor.matmul(psA[:, t * 128:(t + 1) * 128],
                                         xall[:, i, :], cswi[:],
                                         start=True, stop=True)
                    g1w = gpool.tile([64, 4, 128], F16, tag="g1w")
                    nc.scalar.copy(g1w[:, 0:n, :].rearrange("p a k -> p (a k)"),
                                   psA[:, 0:n * 128])
                    ps2 = ps2p.tile([64, 256], F32, tag="ps2")
                    nc.tensor.matmul(ps2[:, 0:n * 64], cmf[:],
                                     g1w[:, 0:n, 0:64], start=True, stop=False)
                    nc.tensor.matmul(ps2[:, 0:n * 64], msf[:],
                                     g1w[:, 0:n, 64:128], start=False, stop=True)
                    p2v = ps2.rearrange("p (k two) -> p k two", two=2)
                    if grp == 0:
                        # halo image: last 4 chunk-cols; imgs 1..3 -> cols 4:100
                        nc.vector.tensor_copy(fbuf[0:64, 0:4], p2v[:, 28:32, 0])
                        nc.vector.tensor_copy(fbuf[64:128, 0:4], p2v[:, 28:32, 1])
                        nc.vector.tensor_copy(fbuf[0:64, 4:100], p2v[:, 32:128, 0])
                        nc.vector.tensor_copy(fbuf[64:128, 4:100],
                                              p2v[:, 32:128, 1])
                    else:
                        base = 4 + (grp * 4 - 1) * 32
                        nc.vector.tensor_copy(fbuf[0:64, base:base + n * 32],
                                              p2v[:, 0:n * 32, 0])
                        nc.vector.tensor_copy(fbuf[64:128, base:base + n * 32],
                                              p2v[:, 0:n * 32, 1])

            # ================= phase 2: conv ================================
            with tc.tile_pool(name="pscv2", bufs=1, space="PSUM") as pscv2:
                psc = [pscv2.tile([128, 512], F32, tag=f"psc{i}",
                                  name=f"psc{i}") for i in range(NBLK)]
                for d in range(4):
                    for blk in range(NBLK):
                        o = 4 + blk * 512 - d
                        nc.tensor.matmul(psc[blk][:], tsb[:, d, :],
                                         fbuf[:, o:o + 512],
                                         start=(d == 0), stop=(d == 3))
                for blk in range(NBLK):
                    nc.vector.tensor_copy(frlin[:, blk * 512:(blk + 1) * 512],
                                          psc[blk][:])

            frl3 = frlin.rearrange("p (b k) -> p b k", b=BS)

            # ======= L1 / L3: acts-stationary 12288->512 + relu + transpose =
            def big_layer(src_blk, d_w, bias_row, out_sb, outT, dma_eng):
                with tc.tile_pool(name="wp", bufs=14) as wp, \
                     tc.tile_pool(name="psm", bufs=1, space="PSUM") as psm, \
                     tc.tile_pool(name="pst", bufs=1, space="PSUM") as pst:
                    acc = psm.tile([32, 512], F32, tag="acc")
                    for K4 in range(24):
                        wt = wp.tile([128, 4, 512], F16, tag="wt")
                        dma_eng.dma_start(wt[:], d_w[K4])
                        for j in range(4):
                            nc.tensor.matmul(acc[:], src_blk(4 * K4 + j),
                                             wt[:, j, :],
                                             start=(K4 == 0 and j == 0),
                                             stop=False)
                    nc.tensor.matmul(acc[:], ones1[:], bias_row[:],
                                     start=False, stop=True)
                    nc.scalar.activation(out_sb[:], acc[:], AF.Relu)
                    pt = pst.tile([128, 128], F16, tag="pt")
                    for fb in range(4):
                        nc.tensor.transpose(pt[:, fb * 32:(fb + 1) * 32],
                                            out_sb[:, fb * 128:(fb + 1) * 128],
                                            idn[:])
                    nc.vector.tensor_copy(
                        outT[:], pt.rearrange("p (f b) -> p f b", f=4))

            big_layer(lambda K: frl3[:, :, K], d_w1, b1s, h_sb, hT, nc.sync)

            # ======= L2 + L3, emission-interleaved ==========================
            # L3's k-block K only needs L2's chunk K//4, and PSUM accumulation
            # is order-independent, so L3's matmuls ride along the L2 loop.
            nc.sync.dma_start(bbig[:], d_b2[:])
            with tc.tile_pool(name="wp2", bufs=4) as wp2, \
                 tc.tile_pool(name="wp3", bufs=3) as wp3, \
                 tc.tile_pool(name="sp2", bufs=3) as sp2, \
                 tc.tile_pool(name="ps2m", bufs=3, space="PSUM") as ps2m, \
                 tc.tile_pool(name="pst2", bufs=3, space="PSUM") as pst2, \
                 tc.tile_pool(name="psm3", bufs=1, space="PSUM") as psm3:
                acc3 = psm3.tile([32, 512], F32, tag="acc3")
                for mc2 in range(12):
                    wt = wp2.tile([128, 4, 1024], F16, tag="w2")
                    nc.sync.dma_start(wt[:], d_w2[mc2])
                    for half in range(2):
                        mc = 2 * mc2 + half
                        acc = ps2m.tile([32, 512], F32, tag="acc2")
                        for fb in range(4):
                            nc.tensor.matmul(
                                acc[:], hT[:, fb, :],
                                wt[:, fb, half * 512:(half + 1) * 512],
                                start=(fb == 0), stop=False)
                        nc.tensor.matmul(acc[:], ones1[:],
                                         bbig[0:1, mc * 512:(mc + 1) * 512],
                                         start=False, stop=True)
                        sb = sp2.tile([32, 512], F16, tag="sb2")
                        if half == 0:
                            nc.scalar.copy(sb[:], acc[:])
                        else:
                            nc.vector.tensor_copy(sb[:], acc[:])
                        pt = pst2.tile([128, 128], F16, tag="pt2")
                        for fb in range(4):
                            nc.tensor.transpose(pt[:, fb * 32:(fb + 1) * 32],
                                                sb[:, fb * 128:(fb + 1) * 128],
                                                idn[:])
                        nc.vector.tensor_copy(
                            specT[:, mc * 4:(mc + 1) * 4, :],
                            pt.rearrange("p (f b) -> p f b", f=4))
                    # L3 portion: k-blocks for the two chunks just produced
                    wt3 = wp3.tile([128, 4, 512], F16, tag="wt3")
                    nc.scalar.dma_start(wt3[:], d_w3[2 * mc2])
                    wt3b = wp3.tile([128, 4, 512], F16, tag="wt3b")
                    nc.scalar.dma_start(wt3b[:], d_w3[2 * mc2 + 1])
                    for K4, w3t in ((2 * mc2, wt3), (2 * mc2 + 1, wt3b)):
                        for j in range(4):
                            nc.tensor.matmul(acc3[:],
                                             specT[:, 4 * K4 + j, :],
                                             w3t[:, j, :],
                                             start=(mc2 == 0 and K4 == 0
                                                    and j == 0),
                                             stop=False)
                nc.tensor.matmul(acc3[:], ones1[:], b3s[:],
                                 start=False, stop=True)
                nc.scalar.activation(h1_sb[:], acc3[:], AF.Relu)
                with tc.tile_pool(name="pst3", bufs=1, space="PSUM") as pst3:
                    pt = pst3.tile([128, 128], F16, tag="pt3")
                    for fb in range(4):
                        nc.tensor.transpose(pt[:, fb * 32:(fb + 1) * 32],
                                            h1_sb[:, fb * 128:(fb + 1) * 128],
                                            idn[:])
                    nc.vector.tensor_copy(
                        h1T[:], pt.rearrange("p (f b) -> p f b", f=4))

            # ======= L4: weights-stationary 512->512 + relu =================
            with tc.tile_pool(name="wp4", bufs=1) as wp4, \
                 tc.tile_pool(name="ps4m", bufs=2, space="PSUM") as ps4m:
                w4 = wp4.tile([128, 4, 512], F16, tag="w4")
                nc.gpsimd.dma_start(w4[:], d_w4.rearrange("a p k -> p a k"))
                for f2b in range(4):
                    acc = ps4m.tile([128, 32], F32, tag="acc4")
                    for fb in range(4):
                        nc.tensor.matmul(acc[:],
                                         w4[:, f2b, fb * 128:(fb + 1) * 128],
                                         h1T[:, fb, :],
                                         start=(fb == 0), stop=(fb == 3))
                    nc.scalar.activation(h2T[:, f2b, :], acc[:], AF.Relu,
                                         bias=b4s[:, f2b:f2b + 1])

            # ======= L5 + ifft2, emission-interleaved by channel ============
            nc.sync.dma_start(bbig[:], d_b5[:])
            with tc.tile_pool(name="wp5", bufs=5) as wp5, \
                 tc.tile_pool(name="sp5", bufs=3) as sp5, \
                 tc.tile_pool(name="opool", bufs=1) as opool, \
                 tc.tile_pool(name="gpi", bufs=2) as gpi, \
                 tc.tile_pool(name="ps5m", bufs=2, space="PSUM") as ps5m, \
                 tc.tile_pool(name="pst5", bufs=2, space="PSUM") as pst5, \
                 tc.tile_pool(name="ps1i", bufs=2, space="PSUM") as ps1i, \
                 tc.tile_pool(name="ps2i", bufs=2, space="PSUM") as ps2i:
                oall = opool.tile([64, NIMG * 64], F32, tag="oall")
                oal3 = oall.rearrange("u (b c v) -> u b c v", b=BS, c=C)

                def ifft2_channel(c):
                    for bg in range(BS // 4):
                        psA = ps1i.tile([64, 512], F32, tag="psAi",
                                        name="psAi")
                        for t in range(4):
                            b = bg * 4 + t
                            nc.tensor.matmul(psA[:, t * 128:(t + 1) * 128],
                                             procTs[c][:, :, b],
                                             cswi[:], start=True, stop=True)
                        g1w = gpi.tile([64, 4, 128], F16, tag="g1i",
                                       name="g1i")
                        nc.scalar.copy(g1w.rearrange("p a k -> p (a k)"),
                                       psA[:])
                        ps2 = ps2i.tile([64, 256], F32, tag="p2i", name="p2i")
                        nc.tensor.matmul(ps2[:], cmi[:], g1w[:, :, 0:64],
                                         start=True, stop=False)
                        nc.tensor.matmul(ps2[:], smi[:], g1w[:, :, 64:128],
                                         start=False, stop=True)
                        nc.scalar.copy(
                            oal3[:, bg * 4:(bg + 1) * 4, c, :],
                            ps2.rearrange("u (b v) -> u b v", b=4))
                        if c == 2:
                            for b0 in (bg * 4, bg * 4 + 2):
                                nc.sync.dma_start(
                                    d_out[b0:b0 + 2].rearrange(
                                        "b c u v -> u b c v"),
                                    oall[:, b0 * 192:(b0 + 2) * 192].rearrange(
                                        "u (b c v) -> u b c v", b=2, c=C))

                for mc2 in range(12):
                    wt = wp5.tile([128, 4, 1024], F16, tag="w5")
                    nc.gpsimd.dma_start(wt[:], d_w5[mc2])
                    for half in range(2):
                        mc = 2 * mc2 + half
                        acc = ps5m.tile([32, 512], F32, tag="acc5")
                        for fb in range(4):
                            nc.tensor.matmul(
                                acc[:], h2T[:, fb, :],
                                wt[:, fb, half * 512:(half + 1) * 512],
                                start=(fb == 0), stop=False)
                        nc.tensor.matmul(acc[:], ones1[:],
                                         bbig[0:1, mc * 512:(mc + 1) * 512],
                                         start=False, stop=True)
                        sb = sp5.tile([32, 512], F16, tag="sb5")
                        if half == 0:
                            nc.scalar.copy(sb[:], acc[:])
                        else:
                            nc.vector.tensor_copy(sb[:], acc[:])
                        pt = pst5.tile([64, 256], F16, tag="pt5")
                        for t in range(8):
                            nc.tensor.transpose(pt[:, t * 32:(t + 1) * 32],
                                                sb[:, t * 64:(t + 1) * 64],
                                                idn[:])
                        nc.vector.tensor_copy(
                            procTs[mc // 8][:, (mc % 8) * 8:(mc % 8 + 1) * 8, :],
                            pt.rearrange("p (t b) -> p t b", t=8))
                    if mc2 in (3, 7, 11):
                        ifft2_channel(mc2 // 4)

    nc.compile()
    return nc


def _get_nc():
    global _NC_CACHE
    if _NC_CACHE is None:
        _NC_CACHE = _build_nc()
    return _NC_CACHE


def _make_in_maps(x, Ws1, bs1, Ws2, bs2, Wn1, bn1, Wn2, bn2, Wn3, bn3):
    shared = dict(_host_constants())
    shared.update(_prep_weights(Ws1, bs1, Ws2, bs2, Wn1, bn1, Wn2, bn2,
                                Wn3, bn3))
    in_maps = []
    for g in range(NCORE):
        if g == 0:
            halo = np.zeros((1, 64, 64), np.float32)
        else:
            halo = x[g * BS - 1, 2][None]
        ximgs = np.concatenate(
            [halo, x[g * BS:(g + 1) * BS].reshape(NIMG, 64, 64),
             np.zeros((1, 64, 64), np.float32)]).astype(np.float16)
        in_maps.append({"ximgs": np.ascontiguousarray(ximgs), **shared})
    return in_maps


def kernel(**inputs):
    x = np.ascontiguousarray(inputs["x"], dtype=np.float32)
    nc = _get_nc()
    in_maps = _make_in_maps(
        x, inputs["Ws1"], inputs["bs1"], inputs["Ws2"], inputs["bs2"],
        inputs["Wn1"], inputs["bn1"], inputs["Wn2"], inputs["bn2"],
        inputs["Wn3"], inputs["bn3"])
    res = run_bass_kernel_spmd(nc, in_maps, list(range(NCORE)))
    out = np.empty((B, C, H, W), np.float32)
    for g in range(NCORE):
        out[g * BS:(g + 1) * BS] = res.results[g]["out"]
    return out


# revision 33
# speedup vs baseline: 585.9883x; 1.0131x over previous
"""Trainium2 Bass kernel for nn_EnhancedFractionalPINO.

Pipeline (per core, batch-parallel over 8 NeuronCores, 32 batches/core):
  1. f = Re(fft2(x)) per 64x64 image via cosine/sine DFT matmuls:
     m1: per image, lhsT = image, rhs = [C | S] -> [x^T C | x^T S];
     m2: per 8-image group, two const-stationary matmuls with strided rhs
     -> A^T = C x^T C - S x^T S for all 8 images in one psum tile.
  2. GL fractional derivative = truncated causal conv (KTAPS taps) over the
     globally-flattened signal, as Toeplitz-block matmuls (halo image passed
     from the previous core's batch range; zeros for core 0). The h^-alpha
     scale is folded into Ws1 so everything stays in fp16 range.
  3. spectral_operator + neural_operator MLPs as fp16 PE matmuls with a
     positive rescaling chain (LAM_*) keeping activations in fp16 range;
     activations-stationary, PE transposes between layers.
  4. out = Re(ifft2(proc)) via the same DFT-matmul machinery (scales folded
     into the second-stage constants).

Weights are replicated across cores; activations stay SBUF-resident.
"""

import numpy as np

import concourse.bass as bass
import concourse.mybir as mybir
import concourse.tile as tile
from concourse import bacc
from concourse.bass_utils import run_bass_kernel_spmd

F32 = mybir.dt.float32
F16 = mybir.dt.float16
AF = mybir.ActivationFunctionType

B, C, H, W = 256, 3, 64, 64
MODES = C * H * W              # 12288
ALPHA = 0.5
NTOT = B * MODES               # 3145728 flattened samples
NCORE = 8
BS = B // NCORE                # 32 batches per core
NIMG = BS * C                  # 96 images per core
NSLOT = NIMG + 2               # halo + 96 images + zero pad
KTAPS = 512                    # truncated GL taps (4 chunks of 128)
NCH = BS * MODES // 128        # 3072 output chunks per core
NBLK = NCH // 512              # 6 conv blocks of 512 chunks

# fp16 activation rescaling chain (see mirror3 validation)
LAM_H, LAM_S, LAM_1, LAM_2, LAM_P = 16.0, 8.0, 4.0, 4.0, 4.0


# ---------------------------------------------------------------- host consts
def _host_constants():
    jk = np.outer(np.arange(64), np.arange(64)).astype(np.float64)
    Cm = np.cos(2 * np.pi * jk / 64)
    Sm = np.sin(2 * np.pi * jk / 64)

    j = np.arange(1, KTAPS, dtype=np.float64)
    w = np.concatenate([[1.0], np.cumprod((j - 1.0 - ALPHA) / j)])

    # Tst[d][t, tau] = w[128*d + tau - t]  (lhsT layout of the Toeplitz blocks)
    idx = 128 * np.arange(4)[:, None, None] \
        + np.arange(128)[None, None, :] - np.arange(128)[None, :, None]
    Tst = np.where((idx >= 0) & (idx < KTAPS), w[np.clip(idx, 0, KTAPS - 1)], 0.0)

    f16 = lambda a: np.ascontiguousarray(a, dtype=np.float16)
    return {
        "cswi": f16(np.concatenate([Cm, Sm], axis=1)),     # [64, 128]
        "cmf": f16(Cm),                                    # [64, 64]
        "msf": f16(-Sm),
        "cmi": f16(Cm * (LAM_P / 4096.0)),
        "smi": f16(-Sm * (LAM_P / 4096.0)),
        "tst": f16(Tst),
        "idn32": f16(np.eye(32)),
        "ones1": f16(np.ones((1, 32))),
    }


def _prep_weights(Ws1, bs1, Ws2, bs2, Wn1, bn1, Wn2, bn2, Wn3, bn3):
    s = float(np.float64(1.0 / (NTOT - 1)) ** (-ALPHA))
    f16 = lambda a: np.ascontiguousarray(a, dtype=np.float16)
    W1 = (Ws1.astype(np.float64) * (s / LAM_H)).astype(np.float32)
    W2 = Ws2 * np.float32(LAM_H / LAM_S)
    W3 = Wn1 * np.float32(LAM_S / LAM_1)
    W4 = Wn2 * np.float32(LAM_1 / LAM_2)
    W5 = Wn3 * np.float32(LAM_2 / LAM_P)
    return {
        "w1t": f16(W1.reshape(24, 4, 128, 512).transpose(0, 2, 1, 3)),
        "w2r": f16(W2.reshape(4, 128, 12, 1024).transpose(2, 1, 0, 3)),
        "w3t": f16(W3.reshape(24, 4, 128, 512).transpose(0, 2, 1, 3)),
        "w4t": f16(W4.reshape(4, 128, 4, 128).transpose(2, 1, 0, 3)
                   .reshape(4, 128, 512)),
        "w5r": f16(W5.reshape(4, 128, 12, 1024).transpose(2, 1, 0, 3)),
        "b1r": f16((bs1 / LAM_H).reshape(1, 512)),
        "b2r": f16((bs2 / LAM_S).reshape(1, MODES)),
        "b3r": f16((bn1 / LAM_1).reshape(1, 512)),
        "b4t": np.ascontiguousarray((bn2 / LAM_2).reshape(4, 128).T,
                                    dtype=np.float32),     # [128, 4]
        "b5r": f16((bn3 / LAM_P).reshape(1, MODES)),
    }


# ---------------------------------------------------------------- bass module
_NC_CACHE = None


def _build_nc():
    nc = bacc.Bacc("TRN2", target_bir_lowering=False, debug=False,
                   num_devices=NCORE)

    def din(name, shape, dt=F16):
        return nc.dram_tensor(name, shape, dt, kind="ExternalInput")

    d_x = din("ximgs", (NSLOT, 64, 64))
    d_cswi = din("cswi", (64, 128))
    d_cmf = din("cmf", (64, 64))
    d_msf = din("msf", (64, 64))
    d_cmi = din("cmi", (64, 64))
    d_smi = din("smi", (64, 64))
    d_tst = din("tst", (4, 128, 128))
    d_idn = din("idn32", (32, 32))
    d_ones = din("ones1", (1, 32))
    d_w1 = din("w1t", (24, 128, 4, 512))
    d_w2 = din("w2r", (12, 128, 4, 1024))
    d_w3 = din("w3t", (24, 128, 4, 512))
    d_w4 = din("w4t", (4, 128, 512))
    d_w5 = din("w5r", (12, 128, 4, 1024))
    d_b1 = din("b1r", (1, 512))
    d_b2 = din("b2r", (1, MODES))
    d_b3 = din("b3r", (1, 512))
    d_b4 = nc.dram_tensor("b4t", (128, 4), F32, kind="ExternalInput")
    d_b5 = din("b5r", (1, MODES))
    d_out = nc.dram_tensor("out", (BS, C, 64, 64), F32, kind="ExternalOutput")

    with tile.TileContext(nc) as tc:
        with tc.tile_pool(name="cpool", bufs=1) as cpool, \
             tc.tile_pool(name="bigpool", bufs=1) as bigpool:
            # ---- constants into SBUF
            cswi = cpool.tile([64, 128], F16, tag="cswi")
            cmf = cpool.tile([64, 64], F16, tag="cmf")
            msf = cpool.tile([64, 64], F16, tag="msf")
            cmi = cpool.tile([64, 64], F16, tag="cmi")
            smi = cpool.tile([64, 64], F16, tag="smi")
            tsb = cpool.tile([128, 4, 128], F16, tag="tsb")
            idn = cpool.tile([32, 32], F16, tag="idn")
            ones1 = cpool.tile([1, 32], F16, tag="ones1")
            b1s = cpool.tile([1, 512], F16, tag="b1s")
            b3s = cpool.tile([1, 512], F16, tag="b3s")
            b4s = cpool.tile([128, 4], F32, tag="b4s")
            bbig = cpool.tile([1, MODES], F16, tag="bbig")  # b2 then b5
            for t, d in ((cswi, d_cswi), (cmf, d_cmf), (msf, d_msf),
                         (cmi, d_cmi), (smi, d_smi), (idn, d_idn),
                         (ones1, d_ones), (b1s, d_b1), (b3s, d_b3),
                         (b4s, d_b4)):
                nc.sync.dma_start(t[:], d[:])
            nc.sync.dma_start(tsb[:], d_tst.rearrange("d p k -> p d k"))

            # ---- persistent activation tiles
            fbuf = bigpool.tile([128, 4 + NCH + 64], F16, tag="fbuf")
            frlin = bigpool.tile([128, NCH], F16, tag="frlin")
            specT = bigpool.tile([128, 96, BS], F16, tag="specT")
            procTs = [bigpool.tile([64, 64, BS], F16, tag=f"procT{i}",
                                   name=f"procT{i}") for i in range(C)]
            hT = bigpool.tile([128, 4, BS], F16, tag="hT")
            h1T = bigpool.tile([128, 4, BS], F16, tag="h1T")
            h2T = bigpool.tile([128, 4, BS], F16, tag="h2T")
            h_sb = bigpool.tile([32, 512], F16, tag="h_sb")
            h1_sb = bigpool.tile([32, 512], F16, tag="h1_sb")

            # ========== phase 1: fft2 (per-image m1, 8-wide m2) =============
            with tc.tile_pool(name="xpool", bufs=1) as xpool, \
                 tc.tile_pool(name="gpool", bufs=6) as gpool, \
                 tc.tile_pool(name="ps1p", bufs=4, space="PSUM") as ps1p, \
                 tc.tile_pool(name="ps2p", bufs=3, space="PSUM") as ps2p:
                xall = xpool.tile([64, NSLOT, 64], F16, tag="xall")
                for ch in range(4):
                    q0 = (NSLOT * ch) // 4
                    q1 = (NSLOT * (ch + 1)) // 4
                    nc.sync.dma_start(
                        xall[:, q0:q1, :],
                        d_x[q0:q1].rearrange("q p k -> p q k"))
                for grp in range(25):
                    n = 4 if grp < 24 else 2
                    psA = ps1p.tile([64, 512], F32, tag="psA")
                    for t in range(n):
                        i = grp * 4 + t
                        nc.tensor.matmul(psA[:, t * 128:(t + 1) * 128],
                                         xall[:, i, :], cswi[:],
                                         start=True, stop=True)
                    g1w = gpool.tile([64, 4, 128], F16, tag="g1w")
                    g1f = g1w[:, 0:n, :].rearrange("p a k -> p (a k)")
                    if grp % 2 == 0:
                        nc.scalar.copy(g1f, psA[:, 0:n * 128])
                    else:
                        nc.vector.tensor_copy(g1f, psA[:, 0:n * 128])
                    ps2 = ps2p.tile([64, 256], F32, tag="ps2")
                    nc.tensor.matmul(ps2[:, 0:n * 64], cmf[:],
                                     g1w[:, 0:n, 0:64], start=True, stop=False)
                    nc.tensor.matmul(ps2[:, 0:n * 64], msf[:],
                                     g1w[:, 0:n, 64:128], start=False, stop=True)
                    p2v = ps2.rearrange("p (k two) -> p k two", two=2)
                    if grp == 0:
                        # halo image: last 4 chunk-cols; imgs 1..3 -> cols 4:100
                        nc.vector.tensor_copy(fbuf[0:64, 0:4], p2v[:, 28:32, 0])
                        nc.vector.tensor_copy(fbuf[64:128, 0:4], p2v[:, 28:32, 1])
                        nc.vector.tensor_copy(fbuf[0:64, 4:100], p2v[:, 32:128, 0])
                        nc.vector.tensor_copy(fbuf[64:128, 4:100],
                                              p2v[:, 32:128, 1])
                    else:
                        base = 4 + (grp * 4 - 1) * 32
                        nc.vector.tensor_copy(fbuf[0:64, base:base + n * 32],
                                              p2v[:, 0:n * 32, 0])
                        nc.vector.tensor_copy(fbuf[64:128, base:base + n * 32],
                                              p2v[:, 0:n * 32, 1])

            # ================= phase 2: conv ================================
            with tc.tile_pool(name="pscv2", bufs=1, space="PSUM") as pscv2:
                psc = [pscv2.tile([128, 512], F32, tag=f"psc{i}",
                                  name=f"psc{i}") for i in range(NBLK)]
                for d in range(4):
                    for blk in range(NBLK):
                        o = 4 + blk * 512 - d
                        nc.tensor.matmul(psc[blk][:], tsb[:, d, :],
                                         fbuf[:, o:o + 512],
                                         start=(d == 0), stop=(d == 3))
                for blk in range(NBLK):
                    nc.vector.tensor_copy(frlin[:, blk * 512:(blk + 1) * 512],
                                          psc[blk][:])

            frl3 = frlin.rearrange("p (b k) -> p b k", b=BS)

            # ======= L1 / L3: acts-stationary 12288->512 + relu + transpose =
            def big_layer(src_blk, d_w, bias_row, out_sb, outT, dma_eng):
                with tc.tile_pool(name="wp", bufs=14) as wp, \
                     tc.tile_pool(name="psm", bufs=1, space="PSUM") as psm, \
                     tc.tile_pool(name="pst", bufs=1, space="PSUM") as pst:
                    acc = psm.tile([32, 512], F32, tag="acc")
                    for K4 in range(24):
                        wt = wp.tile([128, 4, 512], F16, tag="wt")
                        dma_eng.dma_start(wt[:], d_w[K4])
                        for j in range(4):
                            nc.tensor.matmul(acc[:], src_blk(4 * K4 + j),
                                             wt[:, j, :],
                                             start=(K4 == 0 and j == 0),
                                             stop=False)
                    nc.tensor.matmul(acc[:], ones1[:], bias_row[:],
                                     start=False, stop=True)
                    nc.scalar.activation(out_sb[:], acc[:], AF.Relu)
                    pt = pst.tile([128, 128], F16, tag="pt")
                    for fb in range(4):
                        nc.tensor.transpose(pt[:, fb * 32:(fb + 1) * 32],
                                            out_sb[:, fb * 128:(fb + 1) * 128],
                                            idn[:])
                    nc.vector.tensor_copy(
                        outT[:], pt.rearrange("p (f b) -> p f b", f=4))

            big_layer(lambda K: frl3[:, :, K], d_w1, b1s, h_sb, hT, nc.sync)

            # ======= L2 + L3, emission-interleaved ==========================
            # L3's k-block K only needs L2's chunk K//4, and PSUM accumulation
            # is order-independent, so L3's matmuls ride along the L2 loop.
            nc.sync.dma_start(bbig[:], d_b2[:])
            with tc.tile_pool(name="wp2", bufs=4) as wp2, \
                 tc.tile_pool(name="wp3", bufs=3) as wp3, \
                 tc.tile_pool(name="sp2", bufs=3) as sp2, \
                 tc.tile_pool(name="ps2m", bufs=3, space="PSUM") as ps2m, \
                 tc.tile_pool(name="pst2", bufs=3, space="PSUM") as pst2, \
                 tc.tile_pool(name="psm3", bufs=1, space="PSUM") as psm3:
                acc3 = psm3.tile([32, 512], F32, tag="acc3")
                for mc2 in range(12):
                    wt = wp2.tile([128, 4, 1024], F16, tag="w2")
                    nc.sync.dma_start(wt[:], d_w2[mc2])
                    for half in range(2):
                        mc = 2 * mc2 + half
                        acc = ps2m.tile([32, 512], F32, tag="acc2")
                        for fb in range(4):
                            nc.tensor.matmul(
                                acc[:], hT[:, fb, :],
                                wt[:, fb, half * 512:(half + 1) * 512],
                                start=(fb == 0), stop=False)
                        nc.tensor.matmul(acc[:], ones1[:],
                                         bbig[0:1, mc * 512:(mc + 1) * 512],
                                         start=False, stop=True)
                        sb = sp2.tile([32, 512], F16, tag="sb2")
                        if half == 0:
                            nc.scalar.copy(sb[:], acc[:])
                        else:
                            nc.vector.tensor_copy(sb[:], acc[:])
                        pt = pst2.tile([128, 128], F16, tag="pt2")
                        for fb in range(4):
                            nc.tensor.transpose(pt[:, fb * 32:(fb + 1) * 32],
                                                sb[:, fb * 128:(fb + 1) * 128],
                                                idn[:])
                        nc.vector.tensor_copy(
                            specT[:, mc * 4:(mc + 1) * 4, :],
                            pt.rearrange("p (f b) -> p f b", f=4))
                    # L3 portion: k-blocks for the two chunks just produced
                    wt3 = wp3.tile([128, 4, 512], F16, tag="wt3")
                    nc.scalar.dma_start(wt3[:], d_w3[2 * mc2])
                    wt3b = wp3.tile([128, 4, 512], F16, tag="wt3b")
                    nc.scalar.dma_start(wt3b[:], d_w3[2 * mc2 + 1])
                    for K4, w3t in ((2 * mc2, wt3), (2 * mc2 + 1, wt3b)):
                        for j in range(4):
                            nc.tensor.matmul(acc3[:],
                                             specT[:, 4 * K4 + j, :],
                                             w3t[:, j, :],
                                             start=(mc2 == 0 and K4 == 0
                                                    and j == 0),
                                             stop=False)
                nc.tensor.matmul(acc3[:], ones1[:], b3s[:],
                                 start=False, stop=True)
                nc.scalar.activation(h1_sb[:], acc3[:], AF.Relu)
                with tc.tile_pool(name="pst3", bufs=1, space="PSUM") as pst3:
                    pt = pst3.tile([128, 128], F16, tag="pt3")
                    for fb in range(4):
                        nc.tensor.transpose(pt[:, fb * 32:(fb + 1) * 32],
                                            h1_sb[:, fb * 128:(fb + 1) * 128],
                                            idn[:])
                    nc.vector.tensor_copy(
                        h1T[:], pt.rearrange("p (f b) -> p f b", f=4))

            # ======= L4: weights-stationary 512->512 + relu =================
            with tc.tile_pool(name="wp4", bufs=1) as wp4, \
                 tc.tile_pool(name="ps4m", bufs=2, space="PSUM") as ps4m:
                w4 = wp4.tile([128, 4, 512], F16, tag="w4")
                nc.gpsimd.dma_start(w4[:], d_w4.rearrange("a p k -> p a k"))
                for f2b in range(4):
                    acc = ps4m.tile([128, 32], F32, tag="acc4")
                    for fb in range(4):
                        nc.tensor.matmul(acc[:],
                                         w4[:, f2b, fb * 128:(fb + 1) * 128],
                                         h1T[:, fb, :],
                                         start=(fb == 0), stop=(fb == 3))
                    nc.scalar.activation(h2T[:, f2b, :], acc[:], AF.Relu,
                                         bias=b4s[:, f2b:f2b + 1])

            # ======= L5 + ifft2, emission-interleaved by channel ============
            nc.sync.dma_start(bbig[:], d_b5[:])
            with tc.tile_pool(name="wp5", bufs=5) as wp5, \
                 tc.tile_pool(name="sp5", bufs=3) as sp5, \
                 tc.tile_pool(name="opool", bufs=1) as opool, \
                 tc.tile_pool(name="gpi", bufs=2) as gpi, \
                 tc.tile_pool(name="ps5m", bufs=2, space="PSUM") as ps5m, \
                 tc.tile_pool(name="pst5", bufs=2, space="PSUM") as pst5, \
                 tc.tile_pool(name="ps1i", bufs=2, space="PSUM") as ps1i, \
                 tc.tile_pool(name="ps2i", bufs=2, space="PSUM") as ps2i:
                oall = opool.tile([64, NIMG * 64], F32, tag="oall")
                oal3 = oall.rearrange("u (b c v) -> u b c v", b=BS, c=C)

                def ifft2_channel(c):
                    for bg in range(BS // 4):
                        psA = ps1i.tile([64, 512], F32, tag="psAi",
                                        name="psAi")
                        for t in range(4):
                            b = bg * 4 + t
                            nc.tensor.matmul(psA[:, t * 128:(t + 1) * 128],
                                             procTs[c][:, :, b],
                                             cswi[:], start=True, stop=True)
                        g1w = gpi.tile([64, 4, 128], F16, tag="g1i",
                                       name="g1i")
                        if bg % 2 == 0:
                            nc.scalar.copy(g1w.rearrange("p a k -> p (a k)"),
                                           psA[:])
                        else:
                            nc.vector.tensor_copy(
                                g1w.rearrange("p a k -> p (a k)"), psA[:])
                        ps2 = ps2i.tile([64, 256], F32, tag="p2i", name="p2i")
                        nc.tensor.matmul(ps2[:], cmi[:], g1w[:, :, 0:64],
                                         start=True, stop=False)
                        nc.tensor.matmul(ps2[:], smi[:], g1w[:, :, 64:128],
                                         start=False, stop=True)
                        nc.scalar.copy(
                            oal3[:, bg * 4:(bg + 1) * 4, c, :],
                            ps2.rearrange("u (b v) -> u b v", b=4))
                        if c == 2:
                            for b0 in (bg * 4, bg * 4 + 2):
                                nc.sync.dma_start(
                                    d_out[b0:b0 + 2].rearrange(
                                        "b c u v -> u b c v"),
                                    oall[:, b0 * 192:(b0 + 2) * 192].rearrange(
                                        "u (b c v) -> u b c v", b=2, c=C))

                for mc2 in range(12):
                    wt = wp5.tile([128, 4, 1024], F16, tag="w5")
                    nc.gpsimd.dma_start(wt[:], d_w5[mc2])
                    for half in range(2):
                        mc = 2 * mc2 + half
                        acc = ps5m.tile([32, 512], F32, tag="acc5")
                        for fb in range(4):
                            nc.tensor.matmul(
                                acc[:], h2T[:, fb, :],
                                wt[:, fb, half * 512:(half + 1) * 512],
                                start=(fb == 0), stop=False)
                        nc.tensor.matmul(acc[:], ones1[:],
                                         bbig[0:1, mc * 512:(mc + 1) * 512],
                                         start=False, stop=True)
                        sb = sp5.tile([32, 512], F16, tag="sb5")
                        if half == 0:
                            nc.scalar.copy(sb[:], acc[:])
                        else:
                            nc.vector.tensor_copy(sb[:], acc[:])
                        pt = pst5.tile([64, 256], F16, tag="pt5")
                        for t in range(8):
                            nc.tensor.transpose(pt[:, t * 32:(t + 1) * 32],
                                                sb[:, t * 64:(t + 1) * 64],
                                                idn[:])
                        nc.vector.tensor_copy(
                            procTs[mc // 8][:, (mc % 8) * 8:(mc % 8 + 1) * 8, :],
                            pt.rearrange("p (t b) -> p t b", t=8))
                    if mc2 in (3, 7, 11):
                        ifft2_channel(mc2 // 4)

    nc.compile()
    return nc


def _get_nc():
    global _NC_CACHE
    if _NC_CACHE is None:
        _NC_CACHE = _build_nc()
    return _NC_CACHE


def _make_in_maps(x, Ws1, bs1, Ws2, bs2, Wn1, bn1, Wn2, bn2, Wn3, bn3):
    shared = dict(_host_constants())
    shared.update(_prep_weights(Ws1, bs1, Ws2, bs2, Wn1, bn1, Wn2, bn2,
                                Wn3, bn3))
    in_maps = []
    for g in range(NCORE):
        if g == 0:
            halo = np.zeros((1, 64, 64), np.float32)
        else:
            halo = x[g * BS - 1, 2][None]
        ximgs = np.concatenate(
            [halo, x[g * BS:(g + 1) * BS].reshape(NIMG, 64, 64),
             np.zeros((1, 64, 64), np.float32)]).astype(np.float16)
        in_maps.append({"ximgs": np.ascontiguousarray(ximgs), **shared})
    return in_maps


def kernel(**inputs):
    x = np.ascontiguousarray(inputs["x"], dtype=np.float32)
    nc = _get_nc()
    in_maps = _make_in_maps(
        x, inputs["Ws1"], inputs["bs1"], inputs["Ws2"], inputs["bs2"],
        inputs["Wn1"], inputs["bn1"], inputs["Wn2"], inputs["bn2"],
        inputs["Wn3"], inputs["bn3"])
    res = run_bass_kernel_spmd(nc, in_maps, list(range(NCORE)))
    out = np.empty((B, C, H, W), np.float32)
    for g in range(NCORE):
        out[g * BS:(g + 1) * BS] = res.results[g]["out"]
    return out
